# revision 1
# baseline (speedup 1.0000x reference)
"""DiT block kernel for 8x Trainium2 NeuronCores (Bass/Tile).

Sharding: row-parallel over the flattened (B,T)=4096 rows; 512 rows/core.
Cores 0-3 handle batch 0, cores 4-7 batch 1. MQA K/V is computed per-shard
and AllGather'd within each 4-core batch group. Weights are replicated and
cast to bf16 (PE runs bf16 at 1 cycle/row); LN/residual math stays fp32.

Device-side layout notes (per core, R=512 rows):
  - rows-on-partitions for LN/residual tensors (bn_stats reduces over free)
  - hn/h2 are PE-transposed to hT [F-tile, rows] to serve as matmul rhs
  - attention scores are computed transposed ([keys, rows]) so the exp'd
    probs tiles can be used directly as lhsT of the PV matmul; a ones
    column appended to V yields softmax denominators for free; the 1/sum
    is a per-partition scalar on the untransposed PV output.
  - SBUF is tight: phase-0 temporaries and attention-era tiles live in
    pools that are closed as soon as their phase ends.
"""

import os
import sys

sys.path.insert(0, "/opt/trn_rl_repo")

import numpy as np
import ml_dtypes

BF16 = ml_dtypes.bfloat16

B, T, F, H, D, M, C = 2, 2048, 1024, 16, 64, 4, 512
NCORES = 8
R = (B * T) // NCORES  # 512 rows per core
RB = R // 128  # 4 row blocks
FT = F // 128  # 8 feature tiles
MT = (H * D) // 128  # 8 head-pair tiles
MFT = (M * F) // 128  # 32 mlp hidden tiles
KT = T // 128  # 16 key tiles
EPS = 1e-5

_CACHE = {}


def _build_nc():
    import concourse.bass as bass
    import concourse.tile as tile
    from concourse import bacc, mybir
    from concourse.masks import make_identity
    from contextlib import ExitStack

    f32 = mybir.dt.float32
    f16 = mybir.dt.float16
    bf16 = mybir.dt.bfloat16
    AF = mybir.ActivationFunctionType
    OP = mybir.AluOpType

    nc = bacc.Bacc(
        "TRN2",
        target_bir_lowering=False,
        debug=False,
        enable_asserts=False,
        num_devices=NCORES,
    )

    def dram(name, shape, dt, **kw):
        return nc.dram_tensor(name, shape, dt, **kw).ap()

    x_d = dram("x", [R, F], f32, kind="ExternalInput")
    cond_d = dram("cond", [C], bf16, kind="ExternalInput")
    wmod_d = dram("wmod", [C, 4 * F], bf16, kind="ExternalInput")
    modb_d = dram("modb", [4 * F], f32, kind="ExternalInput")
    lnv_d = dram("lnvec", [6, F], f32, kind="ExternalInput")
    wq_d = dram("wq", [MT, 128, FT * 128], bf16, kind="ExternalInput")
    wkv_d = dram("wkv", [F, 2 * D], bf16, kind="ExternalInput")
    wo_d = dram("wo", [H * D, F], bf16, kind="ExternalInput")
    wob_d = dram("wo_bias", [1, F], f32, kind="ExternalInput")
    w1_d = dram("w1", [MFT, 128, FT * 128], bf16, kind="ExternalInput")
    b1_d = dram("b1", [M * F], f32, kind="ExternalInput")
    w2_d = dram("w2", [M * F, F], bf16, kind="ExternalInput")
    b2_d = dram("b2", [1, F], f32, kind="ExternalInput")
    y_d = dram("y", [R, F], f32, kind="ExternalOutput")

    groups = [[0, 1, 2, 3], [4, 5, 6, 7]]

    def bcast_row(ap_row):
        # [1, n] DRAM AP -> partition-broadcast [128, n]
        return bass.AP(
            tensor=ap_row.tensor,
            offset=ap_row.offset,
            ap=[[0, 128]] + list(ap_row.ap[-1:]),
        )

    with tile.TileContext(nc) as tc, ExitStack() as ctx:
        consts = ctx.enter_context(tc.tile_pool(name="consts", bufs=1))
        work = ctx.enter_context(tc.tile_pool(name="work", bufs=2))
        persist = ctx.enter_context(tc.tile_pool(name="persist", bufs=1))
        wstr = ctx.enter_context(tc.tile_pool(name="wstr", bufs=3))
        dramp = ctx.enter_context(tc.tile_pool(name="dramp", bufs=1, space="DRAM"))
        psA = ctx.enter_context(tc.tile_pool(name="psA", bufs=4, space="PSUM"))
        psB = ctx.enter_context(tc.tile_pool(name="psB", bufs=2, space="PSUM"))

        _dmaq_state = [0]

        def dmaq(out, in_):
            # alternate big transfers across the two HWDGE queues
            eng = nc.sync if _dmaq_state[0] % 2 == 0 else nc.scalar
            _dmaq_state[0] += 1
            eng.dma_start(out=out, in_=in_)

        # ---------------- constants ----------------
        ident = consts.tile([128, 128], bf16, name="ident")
        make_identity(nc, ident)
        ones16 = consts.tile([1, 128], f16, name="ones16")
        nc.vector.memset(ones16, 1.0)
        epst = consts.tile([128, 1], f32, name="epst")
        nc.vector.memset(epst, EPS)

        cond_sb = consts.tile([128, 4], bf16, name="cond_sb")
        nc.sync.dma_start(out=cond_sb, in_=cond_d.rearrange("(a p) -> p a", p=128))
        b1_sb = consts.tile([128, MFT], f32, name="b1_sb")
        nc.sync.dma_start(out=b1_sb, in_=b1_d.rearrange("(mt p) -> p mt", p=128))
        wkv_sb = consts.tile([128, FT, 2 * D], bf16, name="wkv_sb")
        nc.sync.dma_start(
            out=wkv_sb, in_=wkv_d.rearrange("(kt p) n -> p kt n", p=128)
        )

        anw_bc = consts.tile([128, F], f32, name="anw_bc")
        nc.sync.dma_start(out=anw_bc, in_=bcast_row(lnv_d[2:3, :]))
        anb_bc = consts.tile([128, F], f32, name="anb_bc")
        nc.sync.dma_start(out=anb_bc, in_=bcast_row(lnv_d[3:4, :]))
        wob_bc = consts.tile([128, F], f32, name="wob_bc")
        nc.sync.dma_start(out=wob_bc, in_=bcast_row(wob_d[0:1, :]))
        b2_bc = consts.tile([128, F], f32, name="b2_bc")
        nc.sync.dma_start(out=b2_bc, in_=bcast_row(b2_d[0:1, :]))

        # ---------------- phase 0: modulation vectors ----------------
        # modv = cond @ [gA | bA | gF | bF] + modb  -> [1, 4F] fp32, then
        # Wa = amod_nw*(1+gA), Ba = amod_nb*(1+gA)+bA (same for fmod),
        # PE-broadcast to [128, F] fp32 tiles.
        cm_modtmp = tc.tile_pool(name="modtmp", bufs=1)
        modtmp = cm_modtmp.__enter__()

        lnr = {}
        for r in (0, 1, 4, 5):  # amod_nw/nb, fmod_nw/nb rows at partition 0
            lnr[r] = modtmp.tile([1, F], f32, name=f"lnr{r}")
            nc.sync.dma_start(out=lnr[r], in_=lnv_d[r : r + 1, :])
        modb_sb = modtmp.tile([1, 4 * F], f32, name="modb_sb")
        nc.sync.dma_start(out=modb_sb, in_=modb_d.rearrange("(a f) -> a f", a=1))
        modv = modtmp.tile([1, 4 * F], f32, name="modv")
        for grp in range(2):  # nb groups of 4 -> 4 concurrent psum accumulators
            wm_tiles = []
            for ch in range(4):
                wm = modtmp.tile(
                    [128, 2048], bf16, tag="wm", bufs=2, name=f"wm{grp}_{ch}"
                )
                dmaq(
                    wm,
                    wmod_d[ch * 128 : (ch + 1) * 128, grp * 2048 : (grp + 1) * 2048],
                )
                wm_tiles.append(wm)
            pms = [
                psA.tile([128, 512], f32, tag="ps", name=f"pm{j}") for j in range(4)
            ]
            for ch in range(4):
                for j in range(4):
                    nc.tensor.matmul(
                        pms[j][0:1, :],
                        cond_sb[:, ch : ch + 1],
                        wm_tiles[ch][:, j * 512 : (j + 1) * 512],
                        start=(ch == 0),
                        stop=(ch == 3),
                    )
            for j in range(4):
                nb = grp * 4 + j
                nc.vector.tensor_add(
                    out=modv[:, nb * 512 : (nb + 1) * 512],
                    in0=pms[j][0:1, :],
                    in1=modb_sb[:, nb * 512 : (nb + 1) * 512],
                )

        # in-place per mod: g-slot := nw*(1+g), b-slot := nb*(1+g) + b, then
        # PE-broadcast. amod (grp0 columns) is finalized first so adaLN-1
        # can start before the fmod half of wmod has even arrived.
        tmpv = modtmp.tile([1, F], f32, name="tmpv")
        bc = {}

        modv16 = modtmp.tile([1, 4 * F], f16, name="modv16")

        def finalize_mod(g_off, b_off, nw_row, nb_row, w_name, b_name):
            g_sl = modv[:, g_off : g_off + F]
            b_sl = modv[:, b_off : b_off + F]
            nc.scalar.add(out=g_sl, in_=g_sl, add=1.0)
            nc.vector.tensor_mul(out=tmpv, in0=g_sl, in1=lnr[nb_row])
            with nc.allow_low_precision(reason="f16 staging for PE broadcast"):
                nc.vector.tensor_add(
                    out=modv16[:, b_off : b_off + F], in0=tmpv, in1=b_sl
                )
                nc.vector.tensor_mul(
                    out=modv16[:, g_off : g_off + F], in0=g_sl, in1=lnr[nw_row]
                )
            for off, nm in ((g_off, w_name), (b_off, b_name)):
                bt = consts.tile([128, F], f32, name=nm)
                for hf in range(2):
                    pb = psA.tile([128, 512], f32, tag="ps", name="pbc")
                    nc.tensor.matmul(
                        pb,
                        ones16,
                        modv16[:, off + hf * 512 : off + (hf + 1) * 512],
                        start=True,
                        stop=True,
                    )
                    nc.scalar.activation(
                        bt[:, hf * 512 : (hf + 1) * 512], pb, AF.Copy
                    )
                bc[nm] = bt

        finalize_mod(0, F, 0, 1, "Wa_bc", "Ba_bc")
        finalize_mod(2 * F, 3 * F, 4, 5, "Wf_bc", "Bf_bc")

        cm_modtmp.__exit__(None, None, None)

        # ---------------- helpers ----------------
        def layer_norm(src, w_bc, b_bc, out_tile):
            """out = LN(src) * w_bc + b_bc ; src [128,F] f32."""
            stats = work.tile([128, 2, 6], f32, tag="stats", name="stats")
            for sg in range(2):
                nc.vector.bn_stats(
                    out=stats[:, sg, :], in_=src[:, sg * 512 : (sg + 1) * 512]
                )
            mv = work.tile([128, 2], f32, tag="mv", name="mv")
            nc.vector.bn_aggr(out=mv, in_=stats)
            rstd = work.tile([128, 1], f32, tag="rstd", name="rstd")
            nc.scalar.activation(
                out=rstd, in_=mv[:, 1:2], func=AF.Sqrt, bias=epst, scale=1.0
            )
            nc.vector.reciprocal(out=rstd, in_=rstd)
            xn = work.tile([128, F], f32, tag="xn", name="xn")
            nc.vector.tensor_scalar(
                out=xn,
                in0=src,
                scalar1=mv[:, 0:1],
                scalar2=rstd,
                op0=OP.subtract,
                op1=OP.mult,
            )
            nc.vector.tensor_mul(out=xn, in0=xn, in1=w_bc)
            # final add on GpSimd frees DVE for the next row-block's stats
            nc.gpsimd.tensor_add(out=out_tile, in0=xn, in1=b_bc)

        def transpose_to(hsrc_bf, hT_tiles, rb):
            """hsrc_bf [128,F] bf16 -> hT_tiles[ft][:, rb*128:+128]."""
            for ft in range(FT):
                pt = psA.tile([128, 128], bf16, tag="ps", name="ptt")
                nc.tensor.transpose(
                    pt, hsrc_bf[:, ft * 128 : (ft + 1) * 128], ident
                )
                nc.scalar.activation(
                    out=hT_tiles[ft][:, rb * 128 : (rb + 1) * 128],
                    in_=pt,
                    func=AF.Copy,
                )

        # hT tags are reused for h2T in phase 6 (same shape/dtype).
        hT = [
            persist.tile([128, R], bf16, tag=f"hT{ft}", name=f"hT{ft}")
            for ft in range(FT)
        ]

        cm_hera = tc.tile_pool(name="hera", bufs=1)
        hera = cm_hera.__enter__()
        cm_aera = tc.tile_pool(name="aera", bufs=1)
        aera = cm_aera.__enter__()
        cm_attnp = tc.tile_pool(name="attnp", bufs=1)
        attnp = cm_attnp.__enter__()

        # ---------------- phase 1: adaLN-1 + attn-LN + transpose ----------------
        h_res = [hera.tile([128, F], f32, name=f"h{rb}") for rb in range(RB)]
        for rb in range(RB):
            x_rb = work.tile([128, F], f32, tag="x", name="x_rb")
            nc.sync.dma_start(out=x_rb, in_=x_d[rb * 128 : (rb + 1) * 128, :])
            layer_norm(x_rb, bc["Wa_bc"], bc["Ba_bc"], h_res[rb])
            hn_bf = work.tile([128, F], bf16, tag="hnbf", bufs=1, name="hn_bf")
            layer_norm(h_res[rb], anw_bc, anb_bc, hn_bf)
            transpose_to(hn_bf, hT, rb)

        # ---------------- phase 2: kv first (AllGather ASAP), then q ----------------
        pkv = psA.tile([128, 512], f32, tag="ps", name="pkv")
        for kt in range(FT):
            nc.tensor.matmul(
                pkv, wkv_sb[:, kt, :], hT[kt], start=(kt == 0), stop=(kt == FT - 1)
            )
        kvT_sb = work.tile([128, R], bf16, tag="kvT", bufs=1, name="kvT_sb")
        nc.scalar.activation(out=kvT_sb, in_=pkv, func=AF.Copy)

        kvT_bounce = dramp.tile([2 * D, R], bf16, name="kvT_bounce")
        kvT_all = dramp.tile([4 * 2 * D, R], bf16, name="kvT_all")
        nc.sync.dma_start(out=kvT_bounce, in_=kvT_sb)
        nc.gpsimd.collective_compute(
            "AllGather",
            OP.bypass,
            replica_groups=groups,
            ins=[kvT_bounce[:, :]],
            outs=[kvT_all[:, :]],
        )

        # q projection fills the AllGather wait
        qT = [aera.tile([128, R], bf16, name=f"qT{mt}") for mt in range(MT)]
        for mt in range(MT):
            wqblk = wstr.tile([128, FT * 128], bf16, tag="wqb", bufs=2, name="wqblk")
            dmaq(out=wqblk, in_=wq_d[mt])
            pq = psA.tile([128, 512], f32, tag="ps", name="pq")
            for kt in range(FT):
                nc.tensor.matmul(
                    pq,
                    wqblk[:, kt * 128 : (kt + 1) * 128],
                    hT[kt],
                    start=(kt == 0),
                    stop=(kt == FT - 1),
                )
            # fold the attention 1/sqrt(D)=0.125 scale into q
            nc.scalar.activation(out=qT[mt], in_=pq, func=AF.Copy, scale=0.125)

        # ---------------- phase 3: kT / v_ext assembly ----------------
        # k^T duplicated into both partition halves so MM1's lhsT can share
        # the rhs (q head slice) base partition for even and odd heads.
        kT = aera.tile([128, T], bf16, name="kT")
        for hp in (0, 64):
            for r in range(4):
                nc.sync.dma_start(
                    out=kT[hp : hp + 64, r * R : (r + 1) * R],
                    in_=kvT_all[r * 128 : r * 128 + 64, :],
                )
        v_ext = [aera.tile([128, 65], bf16, name=f"vext{kt}") for kt in range(KT)]
        for kt in range(KT):
            nc.vector.memset(v_ext[kt][:, 64:65], 1.0)
        for r in range(4):
            vT_sb = work.tile([64, R], bf16, tag="vTs", bufs=1, name="vT_sb")
            nc.sync.dma_start(
                out=vT_sb, in_=kvT_all[r * 128 + 64 : (r + 1) * 128, :]
            )
            for cc in range(4):
                ptv = psA.tile([128, 128], bf16, tag="ps", name="ptv")
                nc.tensor.matmul(
                    ptv[:, 0:64],
                    vT_sb[:, cc * 128 : (cc + 1) * 128],
                    ident[0:64, 0:64],
                    is_transpose=True,
                )
                nc.scalar.activation(
                    out=v_ext[r * 4 + cc][:, 0:64], in_=ptv[:, 0:64], func=AF.Copy
                )

        # ---------------- phase 4: attention ----------------
        # ones row at partition 64 for the denominator-broadcast matmul
        # (matmul operand base partitions must match; psum sums sit at 64).
        ones64 = consts.tile([128, 64], f16, name="ones64")
        nc.vector.memset(ones64[64:65, :], 1.0)
        outT = [aera.tile([64, R], bf16, name=f"outTh{h}") for h in range(H)]

        def mm1_half(hi, lo, hi_kt, prs):
            """QK^T + exp for key tiles [lo, hi_kt) of head hi."""
            mt, hp = hi // 2, (hi % 2) * 64
            for kt in range(lo, hi_kt):
                ps_s = psA.tile([128, 512], f32, tag="ps", name="ps_s")
                nc.tensor.matmul(
                    ps_s,
                    kT[hp : hp + 64, kt * 128 : (kt + 1) * 128],
                    qT[mt][hp : hp + 64, :],
                    start=True,
                    stop=True,
                )
                pr = attnp.tile(
                    [128, R], bf16, tag=f"pr{kt}", bufs=2 if kt < 8 else 1,
                    name=f"pr{kt}",
                )
                nc.scalar.activation(out=pr, in_=ps_s, func=AF.Exp)
                prs[kt] = pr

        # software pipeline: kt 0..7 of head h+1 (double-buffered probs) are
        # issued ahead; kt 8..15 (single-buffered) after the previous head's
        # PV matmul has consumed them.
        probs_cur: dict = {}
        probs_nxt: dict = {}
        mm1_half(0, 0, 8, probs_cur)
        for hi in range(H):
            probs = probs_cur
            mt, hp = hi // 2, (hi % 2) * 64
            # PV matmul, transposed: out^T[65, rows] accumulated over key
            # tiles; row 64 is the softmax denominator (ones column of v).
            po = psB.tile([128, 512], f32, tag="pb", name="po")
            for kt in range(8):
                nc.tensor.matmul(
                    po[0:65, :],
                    v_ext[kt][:, 0:65],
                    probs[kt],
                    start=(kt == 0),
                    stop=False,
                )
            mm1_half(hi, 8, KT, probs_cur)
            for kt in range(8, KT):
                nc.tensor.matmul(
                    po[0:65, :],
                    v_ext[kt][:, 0:65],
                    probs[kt],
                    start=False,
                    stop=(kt == KT - 1),
                )
            rcp_row = work.tile([128, R], f16, tag="rcp", bufs=1, name="rcp_row")
            with nc.allow_low_precision(reason="f16 softmax denom broadcast"):
                nc.vector.reciprocal(out=rcp_row[64:65, :], in_=po[64:65, :])
            bcr = psB.tile([128, 512], f32, tag="pb", name="bcr")
            nc.tensor.matmul(
                bcr[0:64, :],
                ones64[64:65, :],
                rcp_row[64:65, :],
                start=True,
                stop=True,
            )
            t_sb = work.tile([64, R], bf16, tag="tsb", name="t_sb")
            nc.vector.tensor_copy(out=t_sb, in_=po[0:64, :])
            nc.vector.tensor_mul(out=outT[hi], in0=t_sb, in1=bcr[0:64, :])
            if hi + 1 < H:
                probs_nxt = {}
                mm1_half(hi + 1, 0, 8, probs_nxt)
                probs_cur = probs_nxt

        # ---------------- phase 5: out proj + residual -> x1 ----------------
        x1 = [persist.tile([128, F], f32, name=f"x1_{rt}") for rt in range(RB)]
        for rh in range(2):
            px1 = {}
            for rt in (2 * rh, 2 * rh + 1):
                px1[rt] = psB.tile([128, F], f32, tag="pb", name=f"px1_{rt}")
            for hk in range(H):
                woc = wstr.tile([64, F], bf16, tag="woc", bufs=2, name="woc")
                dmaq(woc, wo_d[hk * 64 : (hk + 1) * 64, :])
                for rt in (2 * rh, 2 * rh + 1):
                    for nh in range(2):
                        nc.tensor.matmul(
                            px1[rt][:, nh * 512 : (nh + 1) * 512],
                            outT[hk][:, rt * 128 : (rt + 1) * 128],
                            woc[:, nh * 512 : (nh + 1) * 512],
                            start=(hk == 0),
                            stop=(hk == H - 1),
                        )
            for rt in (2 * rh, 2 * rh + 1):
                nc.vector.tensor_add(out=x1[rt], in0=px1[rt], in1=h_res[rt])
                nc.vector.tensor_add(out=x1[rt], in0=x1[rt], in1=wob_bc)

        cm_attnp.__exit__(None, None, None)
        cm_aera.__exit__(None, None, None)
        cm_hera.__exit__(None, None, None)

        # ---------------- phase 6: adaLN-2 + transpose ----------------
        h2T = [
            persist.tile([128, R], bf16, tag=f"hT{ft}", name=f"h2T{ft}")
            for ft in range(FT)
        ]
        for rt in range(RB):
            h2_bf = work.tile([128, F], bf16, tag="hnbf", bufs=1, name="h2_bf")
            layer_norm(x1[rt], bc["Wf_bc"], bc["Bf_bc"], h2_bf)
            transpose_to(h2_bf, h2T, rt)

        # ---------------- phase 7: mlp1 + gelu ----------------
        g1T = [persist.tile([128, R], bf16, name=f"g1T{mt}") for mt in range(MFT)]
        for mt in range(MFT):
            w1blk = wstr.tile([128, FT * 128], bf16, tag="w1b", bufs=3, name="w1blk")
            dmaq(out=w1blk, in_=w1_d[mt])
            pg = psA.tile([128, 512], f32, tag="ps", name="pg")
            for kt in range(FT):
                nc.tensor.matmul(
                    pg,
                    w1blk[:, kt * 128 : (kt + 1) * 128],
                    h2T[kt],
                    start=(kt == 0),
                    stop=(kt == FT - 1),
                )
            nc.scalar.activation(
                out=g1T[mt],
                in_=pg,
                func=AF.Gelu,
                bias=b1_sb[:, mt : mt + 1],
                scale=1.0,
            )

        # ---------------- phase 8: mlp2 + residual -> y ----------------
        # F split in half; 4 row-tile accumulators live in psA; w2 is read
        # exactly once (each half-column sweep reads its half of every chunk).
        for fh in range(2):
            pf = {}
            for rt in range(RB):
                pf[rt] = psA.tile([128, 512], f32, tag="ps", name=f"pf{rt}")
            for kt in range(MFT):
                w2c = wstr.tile([128, 512], bf16, tag="w2c", bufs=3, name="w2c")
                dmaq(w2c, w2_d[kt * 128 : (kt + 1) * 128, fh * 512 : (fh + 1) * 512])
                for rt in range(RB):
                    nc.tensor.matmul(
                        pf[rt],
                        g1T[kt][:, rt * 128 : (rt + 1) * 128],
                        w2c,
                        start=(kt == 0),
                        stop=(kt == MFT - 1),
                    )
            for rt in range(RB):
                sl = slice(fh * 512, (fh + 1) * 512)
                yh = work.tile([128, 512], f32, tag="yh", bufs=2, name="yh")
                nc.vector.tensor_add(out=yh, in0=pf[rt], in1=x1[rt][:, sl])
                nc.vector.tensor_add(out=yh, in0=yh, in1=b2_bc[:, sl])
                nc.sync.dma_start(out=y_d[rt * 128 : (rt + 1) * 128, sl], in_=yh)

    nc.compile()
    return nc


def _prep_in_maps(inputs):
    f32 = np.float32
    wmod = np.concatenate(
        [inputs["amod_gw"], inputs["amod_bw"], inputs["fmod_gw"], inputs["fmod_bw"]],
        axis=1,
    ).astype(BF16)
    modb = np.concatenate(
        [inputs["amod_gb"], inputs["amod_bb"], inputs["fmod_gb"], inputs["fmod_bb"]]
    ).astype(f32)
    lnvec = np.stack(
        [
            inputs["amod_nw"],
            inputs["amod_nb"],
            inputs["attn_nw"],
            inputs["attn_nb"],
            inputs["fmod_nw"],
            inputs["fmod_nb"],
        ]
    ).astype(f32)
    wq_t = np.ascontiguousarray(
        np.asarray(inputs["wq"]).astype(BF16).reshape(FT, 128, MT, 128)
        .transpose(2, 1, 0, 3).reshape(MT, 128, FT * 128)
    )
    w1_t = np.ascontiguousarray(
        np.asarray(inputs["w1"]).astype(BF16).reshape(FT, 128, MFT, 128)
        .transpose(2, 1, 0, 3).reshape(MFT, 128, FT * 128)
    )
    shared = dict(
        wmod=wmod,
        modb=modb,
        lnvec=lnvec,
        wq=wq_t,
        wkv=np.asarray(inputs["wkv"]).astype(BF16),
        wo=np.asarray(inputs["wo"]).astype(BF16),
        wo_bias=np.asarray(inputs["wo_b"]).astype(f32).reshape(1, F),
        w1=w1_t,
        b1=np.asarray(inputs["b1"]).astype(f32),
        w2=np.asarray(inputs["w2"]).astype(BF16),
        b2=np.asarray(inputs["b2"]).astype(f32).reshape(1, F),
    )
    x = np.asarray(inputs["x"]).astype(f32)
    cond = np.asarray(inputs["cond"]).astype(BF16)
    in_maps = []
    for c in range(NCORES):
        b, r0 = c // 4, (c % 4) * R
        m = dict(shared)
        m["x"] = np.ascontiguousarray(x[b, r0 : r0 + R, :])
        m["cond"] = np.ascontiguousarray(cond[b])
        in_maps.append(m)
    return in_maps


def _run(inputs, trace=False):
    from concourse.bass_utils import run_bass_kernel_spmd

    if "nc" not in _CACHE:
        _CACHE["nc"] = _build_nc()
    nc = _CACHE["nc"]
    in_maps = _prep_in_maps(inputs)
    res = run_bass_kernel_spmd(
        nc, in_maps, core_ids=list(range(NCORES)), trace=trace
    )
    y = np.empty((B, T, F), np.float32)
    for c in range(NCORES):
        b, r0 = c // 4, (c % 4) * R
        y[b, r0 : r0 + R, :] = res.results[c]["y"]
    return y, res


def kernel(**inputs) -> np.ndarray:
    y, _ = _run(inputs, trace=False)
    return y


if __name__ == "__main__":
    _build_nc()
    print("build OK")



# revision 10
# speedup vs baseline: 1.6851x; 1.6851x over previous
"""DiT block kernel for 8x Trainium2 NeuronCores (Bass/Tile).

Sharding: row-parallel over the flattened (B,T)=4096 rows; 512 rows/core.
Cores 0-3 handle batch 0, cores 4-7 batch 1. MQA K/V is computed per-shard
and AllGather'd within each 4-core batch group. Weights are replicated and
cast to bf16; LN/residual math stays fp32.

v2 structure notes (driven by trace analysis of v1):
  - ALL attention matmuls run in the PE's (128,128) tile mode: QK^T uses
    zero-padded kT tiles (kT_lo has k on partitions 0-63 and zeros above,
    kT_hi the reverse) so the contraction is always 128-wide.  v1 alternated
    (64,128) MM1s with (128,128) PV matmuls instruction-by-instruction and
    every matmul paid a PE mode-switch drain (~2.5x slowdown).
  - exp() is applied to [128,1024] PSUM chunks (2 key tiles at once) to
    amortize the ~370ns ACT SBUF/PSUM access latency per instruction.
  - softmax denominators are gathered (via tiny PSUM->SBUF DMAs) into one
    [16,512] tile and inverted with ONE DVE reciprocal (v1: 16 calls at
    2.4us each), then broadcast per head-pair with a single (128,128)-mode
    select-matmul.
  - attention outputs land directly in head-pair-stacked tiles [128,R]
    (even head on partitions 0-63, odd on 64-127, via a dual-layout v_ext)
    so the out-projection contracts 128 partitions per matmul (v1: 64).
  - the adaLN-2 / attn-LN scale+bias are folded into the PSUM->SBUF copies
    of the transposes (per-partition scale/bias APs on ACT).
  - mod vectors, LN chains, DMA queue placement all restructured so the
    K/V AllGather is issued ~30us into the kernel instead of ~125us.
"""

import sys

sys.path.insert(0, "/opt/trn_rl_repo")

import numpy as np
import ml_dtypes

BF16 = ml_dtypes.bfloat16

B, T, F, H, D, M, C = 2, 2048, 1024, 16, 64, 4, 512
NCORES = 8
R = (B * T) // NCORES  # 512 rows per core
RB = R // 128  # 4 row blocks
FT = F // 128  # 8 feature tiles
MT = (H * D) // 128  # 8 head-pair tiles
MFT = (M * F) // 128  # 32 mlp hidden tiles
KT = T // 128  # 16 key tiles
EPS = 1e-5

_CACHE = {}


def _build_nc():
    import concourse.bass as bass
    import concourse.tile as tile
    from concourse import bacc, mybir
    from concourse.masks import make_identity
    from contextlib import ExitStack

    f32 = mybir.dt.float32
    f16 = mybir.dt.float16
    bf16 = mybir.dt.bfloat16
    AF = mybir.ActivationFunctionType
    OP = mybir.AluOpType

    nc = bacc.Bacc(
        "TRN2",
        target_bir_lowering=False,
        debug=False,
        enable_asserts=False,
        num_devices=NCORES,
    )

    def dram(name, shape, dt, **kw):
        return nc.dram_tensor(name, shape, dt, **kw).ap()

    x_d = dram("x", [R, F], f32, kind="ExternalInput")
    cond_d = dram("cond", [C], bf16, kind="ExternalInput")
    wmod_d = dram("wmod", [C, 4 * F], bf16, kind="ExternalInput")
    modb_d = dram("modb", [4 * F], f32, kind="ExternalInput")
    lnv_d = dram("lnvec", [6, F], f32, kind="ExternalInput")
    wq_d = dram("wq", [MT, 128, FT * 128], bf16, kind="ExternalInput")
    wkv_d = dram("wkv", [F, 2 * D], bf16, kind="ExternalInput")
    wo_d = dram("wo", [H * D, F], bf16, kind="ExternalInput")
    wob_d = dram("wo_bias", [1, F], f32, kind="ExternalInput")
    w1_d = dram("w1", [MFT, 128, FT * 128], bf16, kind="ExternalInput")
    b1_d = dram("b1", [M * F], f32, kind="ExternalInput")
    w2_d = dram("w2", [M * F, F], bf16, kind="ExternalInput")
    b2_d = dram("b2", [1, F], f32, kind="ExternalInput")
    sel2_d = dram("sel2", [128, H * 64], bf16, kind="ExternalInput")
    y_d = dram("y", [R, F], f32, kind="ExternalOutput")

    groups = [[0, 1, 2, 3], [4, 5, 6, 7]]

    def bcast_row(ap_row):
        # [1, n] DRAM AP -> partition-broadcast [128, n]
        return bass.AP(
            tensor=ap_row.tensor,
            offset=ap_row.offset,
            ap=[[0, 128]] + list(ap_row.ap[-1:]),
        )

    def row_cols(ap2d, r):
        # row r of a [*, F] DRAM AP, viewed as [128, FT] columns:
        # out[p, t] = row[t*128 + p]
        row = bass.AP(
            tensor=ap2d.tensor,
            offset=ap2d.offset + r * F,
            ap=[[1, F]],
        )
        return row.rearrange("(t p) -> p t", p=128)

    with tile.TileContext(nc) as tc, ExitStack() as ctx:
        consts = ctx.enter_context(tc.tile_pool(name="consts", bufs=1))
        work = ctx.enter_context(tc.tile_pool(name="work", bufs=2))
        persist = ctx.enter_context(tc.tile_pool(name="persist", bufs=1))
        wstr = ctx.enter_context(tc.tile_pool(name="wstr", bufs=3))
        dramp = ctx.enter_context(tc.tile_pool(name="dramp", bufs=1, space="DRAM"))

        # ---------------- constants ----------------
        ident = consts.tile([128, 128], bf16, name="ident")
        make_identity(nc, ident)
        epst = consts.tile([128, 1], f32, name="epst")
        nc.vector.memset(epst, EPS)

        cond_sb = consts.tile([128, 4], bf16, name="cond_sb")
        nc.scalar.dma_start(out=cond_sb, in_=cond_d.rearrange("(a p) -> p a", p=128))
        b1_sb = consts.tile([128, MFT], f32, name="b1_sb")
        nc.scalar.dma_start(out=b1_sb, in_=b1_d.rearrange("(mt p) -> p mt", p=128))
        wkv_sb = consts.tile([128, FT, 2 * D], bf16, name="wkv_sb")
        nc.scalar.dma_start(
            out=wkv_sb, in_=wkv_d.rearrange("(kt p) n -> p kt n", p=128)
        )
        # attn-LN weight/bias in column layout (per-partition scale/bias of
        # the transpose copies)
        anwT = consts.tile([128, FT], f32, name="anwT")
        nc.scalar.dma_start(out=anwT, in_=row_cols(lnv_d, 2))
        anbT = consts.tile([128, FT], f32, name="anbT")
        nc.scalar.dma_start(out=anbT, in_=row_cols(lnv_d, 3))
        wob_bc = consts.tile([128, F], f32, name="wob_bc")
        nc.scalar.dma_start(out=wob_bc, in_=bcast_row(wob_d[0:1, :]))
        b2_bc = consts.tile([128, F], f32, name="b2_bc")
        nc.scalar.dma_start(out=b2_bc, in_=bcast_row(b2_d[0:1, :]))

        # head-pair select matrix for the softmax-denominator broadcast
        # (host-built: block hp2 selects den row 2*hp2 into output partitions
        # 0-63 and row 2*hp2+1 into partitions 64-127)
        sel2 = consts.tile([128, H * 64], bf16, name="sel2")
        nc.scalar.dma_start(out=sel2, in_=sel2_d)

        # phase-scoped SBUF pools, opened in LIFO-compatible nesting order
        # (innermost closes first): hera > attnp > midp > [psum scopes]
        cm_hera = tc.tile_pool(name="hera", bufs=1)  # hT
        hera = cm_hera.__enter__()
        cm_attn = tc.tile_pool(name="attnp", bufs=1)  # attention-era tiles
        attnp = cm_attn.__enter__()
        cm_mid = tc.tile_pool(name="midp", bufs=1)  # h_res/out2/wo2 era
        midp = cm_mid.__enter__()

        # PSUM pool for the startup phases (mod / transposes / kv / q / v_ext)
        cm_ps_start = tc.tile_pool(name="ps_start", bufs=1, space="PSUM")
        ps_start = cm_ps_start.__enter__()

        # ---------------- phase 0: modulation vectors ----------------
        # modv = cond @ [gA | bA | gF | bF] + modb, kept in a row-chunked
        # [8, 512] layout (chunk nb on partition nb) so the finalize math
        # runs on multiple DVE lanes and SBUF address-space cost stays tiny.
        # Wa/Ba are then round-tripped through DRAM into partition-broadcast
        # [128,F] tiles; Wf/Bf into column [128,FT] tiles.
        cm_modtmp = tc.tile_pool(name="modtmp", bufs=1)
        modtmp = cm_modtmp.__enter__()

        def row_chunked(ap2d, r, p):
            # row r of a [*, F-or-4F] DRAM AP as [p, n/p] (chunk c on lane c)
            n = ap2d.ap[-1][1]
            row = bass.AP(
                tensor=ap2d.tensor,
                offset=ap2d.offset + r * n,
                ap=[[1, n]],
            )
            return row.rearrange("(p j) -> p j", p=p)

        lnr8 = {}
        for r in (0, 1, 4, 5):  # amod_nw/nb, fmod_nw/nb as [2, 512]
            lnr8[r] = modtmp.tile([2, 512], f32, name=f"lnr8_{r}")
            nc.scalar.dma_start(out=lnr8[r], in_=row_chunked(lnv_d, r, 2))
        # four [2,512] vectors (gA, bA, gF, bF), each at base partition 0
        # (compute engines require base partition in {0,32,64,96})
        mvec = [modtmp.tile([2, 512], f32, name=f"mvec{v}") for v in range(4)]
        mbias = [modtmp.tile([2, 512], f32, name=f"mbias{v}") for v in range(4)]
        for v in range(4):
            nc.scalar.dma_start(
                out=mbias[v],
                in_=row_chunked(modb_d.rearrange("(a f) -> a f", a=4), v, 2),
            )
        for grp in range(2):
            for j in range(4):
                nb = grp * 4 + j
                pm = ps_start.tile([128, 512], f32, tag="sp", bufs=3, name="pm")
                for ch in range(4):
                    wm = modtmp.tile(
                        [128, 512], bf16, tag="wm", bufs=4, name="wm"
                    )
                    eng = nc.sync if ch % 2 == 0 else nc.gpsimd
                    eng.dma_start(
                        out=wm,
                        in_=wmod_d[
                            ch * 128 : (ch + 1) * 128,
                            nb * 512 : (nb + 1) * 512,
                        ],
                    )
                    nc.tensor.matmul(
                        pm[0:1, :],
                        cond_sb[:, ch : ch + 1],
                        wm,
                        start=(ch == 0),
                        stop=(ch == 3),
                    )
                pmrow = modtmp.tile([1, 512], f32, tag="pmrow", bufs=2, name="pmrow")
                nc.scalar.activation(out=pmrow, in_=pm[0:1, :], func=AF.Copy)
                # relocate chunk nb to lane nb%2 of vector nb//2
                nc.sync.dma_start(
                    out=mvec[nb // 2][nb % 2 : nb % 2 + 1, :], in_=pmrow
                )
        for v in range(4):
            nc.vector.tensor_add(out=mvec[v], in0=mvec[v], in1=mbias[v])

        # finalize: g := nw*(1+g), b := nb*(1+g) + b
        tmp2 = modtmp.tile([2, 512], f32, name="tmp2")
        for gi, bi, nw_r, nb_r in ((0, 1, 0, 1), (2, 3, 4, 5)):
            nc.scalar.add(out=mvec[gi], in_=mvec[gi], add=1.0)
            nc.vector.tensor_mul(out=tmp2, in0=mvec[gi], in1=lnr8[nb_r])
            nc.vector.tensor_add(out=mvec[bi], in0=tmp2, in1=mvec[bi])
            nc.vector.tensor_mul(out=mvec[gi], in0=mvec[gi], in1=lnr8[nw_r])

        # bounce through DRAM: amod rows -> [128,F] broadcast tiles,
        # fmod rows -> [128,FT] column tiles
        mod_dram = dramp.tile([4, F], f32, name="mod_dram")
        for v in range(4):
            nc.sync.dma_start(
                out=row_chunked(mod_dram, v, 2), in_=mvec[v]
            )
        bc = {}
        for v, nm in ((0, "Wa_bc"), (1, "Ba_bc")):
            bt = consts.tile([128, F], f32, name=nm)
            nc.sync.dma_start(out=bt, in_=bcast_row(mod_dram[v : v + 1, :]))
            bc[nm] = bt
        WfC = consts.tile([128, FT], f32, name="WfC")
        nc.sync.dma_start(out=WfC, in_=row_cols(mod_dram, 2))
        BfC = consts.tile([128, FT], f32, name="BfC")
        nc.sync.dma_start(out=BfC, in_=row_cols(mod_dram, 3))

        cm_modtmp.__exit__(None, None, None)

        # ---------------- LN stats helper ----------------
        def ln_stats(src):
            stats = work.tile([128, 2, 6], f32, tag="stats", name="stats")
            for sg in range(2):
                nc.vector.bn_stats(
                    out=stats[:, sg, :], in_=src[:, sg * 512 : (sg + 1) * 512]
                )
            mv = work.tile([128, 2], f32, tag="mv", name="mv")
            nc.vector.bn_aggr(out=mv, in_=stats)
            rstd = work.tile([128, 1], f32, tag="rstd", name="rstd")
            nc.scalar.activation(
                out=rstd, in_=mv[:, 1:2], func=AF.Sqrt, bias=epst, scale=1.0
            )
            nc.vector.reciprocal(out=rstd, in_=rstd)
            return mv, rstd

        hT = [hera.tile([128, R], bf16, name=f"hT{ft}") for ft in range(FT)]
        h_res = [midp.tile([128, F], bf16, name=f"h{rb}") for rb in range(RB)]

        # ---------------- phase 1: adaLN-1 + attn-LN + transpose ----------------
        for rb in range(RB):
            x_rb = work.tile([128, F], f32, tag="x", name="x_rb")
            nc.sync.dma_start(out=x_rb, in_=x_d[rb * 128 : (rb + 1) * 128, :])
            mv1, rstd1 = ln_stats(x_rb)
            nc.vector.scalar_tensor_tensor(
                out=x_rb,
                in0=x_rb,
                scalar=mv1[:, 0:1],
                in1=bc["Wa_bc"],
                op0=OP.subtract,
                op1=OP.mult,
            )
            nc.vector.scalar_tensor_tensor(
                out=h_res[rb],
                in0=x_rb,
                scalar=rstd1,
                in1=bc["Ba_bc"],
                op0=OP.mult,
                op1=OP.add,
            )
            mv2, rstd2 = ln_stats(h_res[rb])
            xn_bf = work.tile([128, F], bf16, tag="xn", name="xn_bf")
            nc.vector.tensor_scalar(
                out=xn_bf,
                in0=h_res[rb],
                scalar1=mv2[:, 0:1],
                scalar2=rstd2,
                op0=OP.subtract,
                op1=OP.mult,
            )
            for ft in range(FT):
                pt = ps_start.tile([128, 128], bf16, tag="tp", bufs=2, name="pt")
                nc.tensor.transpose(pt, xn_bf[:, ft * 128 : (ft + 1) * 128], ident)
                nc.scalar.activation(
                    out=hT[ft][:, rb * 128 : (rb + 1) * 128],
                    in_=pt,
                    func=AF.Identity,
                    bias=anbT[:, ft : ft + 1],
                    scale=anwT[:, ft : ft + 1],
                )

        # ---------------- phase 2: kv first (AllGather ASAP), then q ----------------
        pkv = ps_start.tile([128, 512], f32, tag="sp", bufs=3, name="pkv")
        for kt in range(FT):
            nc.tensor.matmul(
                pkv, wkv_sb[:, kt, :], hT[kt], start=(kt == 0), stop=(kt == FT - 1)
            )
        kvT_sb = work.tile([128, R], bf16, tag="kvT", bufs=1, name="kvT_sb")
        nc.scalar.activation(out=kvT_sb, in_=pkv, func=AF.Copy)

        kvT_bounce = dramp.tile([2 * D, R], bf16, name="kvT_bounce")
        kvT_all = dramp.tile([4 * 2 * D, R], bf16, name="kvT_all")
        nc.sync.dma_start(out=kvT_bounce, in_=kvT_sb)
        nc.gpsimd.collective_compute(
            "AllGather",
            OP.bypass,
            replica_groups=groups,
            ins=[kvT_bounce[:, :]],
            outs=[kvT_all[:, :]],
        )

        # q projection fills the AllGather wait
        qT = [attnp.tile([128, R], bf16, name=f"qT{mt}") for mt in range(MT)]
        for mt in range(MT):
            wqblk = wstr.tile([128, FT * 128], bf16, tag="wqb", bufs=3, name="wqblk")
            nc.sync.dma_start(out=wqblk, in_=wq_d[mt])
            pq = ps_start.tile([128, 512], f32, tag="sp", bufs=3, name="pq")
            for kt in range(FT):
                nc.tensor.matmul(
                    pq,
                    wqblk[:, kt * 128 : (kt + 1) * 128],
                    hT[kt],
                    start=(kt == 0),
                    stop=(kt == FT - 1),
                )
            # fold the attention 1/sqrt(D)=0.125 scale into q
            nc.scalar.activation(out=qT[mt], in_=pq, func=AF.Copy, scale=0.125)

        # preload exp's ACT table set during the AllGather window
        warm = work.tile([1, 1], f32, tag="warm", bufs=1, name="warm")
        nc.scalar.activation(out=warm, in_=epst[0:1, 0:1], func=AF.Exp)

        # prefetch wo (consumed in phase 5)
        wo2sb = [midp.tile([128, F], bf16, name=f"wo2_{hp}") for hp in range(MT)]
        for hp in range(MT):
            nc.gpsimd.dma_start(out=wo2sb[hp], in_=wo_d[hp * 128 : (hp + 1) * 128, :])

        # ---------------- phase 3: kT / v_ext assembly ----------------
        # kT_lo: k on partitions 0-63, zeros above  -> even heads (q on 0-63)
        # kT_hi: k on partitions 64-127, zeros below -> odd heads
        # Both give contraction-128 matmuls: no PE tile-mode switching.
        kT_lo = attnp.tile([128, T], bf16, name="kT_lo")
        kT_hi = attnp.tile([128, T], bf16, name="kT_hi")
        nc.vector.memset(kT_lo[64:128, :], 0.0)
        nc.vector.memset(kT_hi[0:64, :], 0.0)
        for r in range(4):
            nc.sync.dma_start(
                out=kT_lo[0:64, r * R : (r + 1) * R],
                in_=kvT_all[r * 128 : r * 128 + 64, :],
            )
            nc.gpsimd.dma_start(
                out=kT_hi[64:128, r * R : (r + 1) * R],
                in_=kvT_all[r * 128 : r * 128 + 64, :],
            )
        # v_ext dual layout [128, 192]:
        #   cols 0-63 = v dims, col 64 = ones, cols 65-127 = 0, cols 128-191 = v dims
        # even heads use lhsT [:,0:65]  -> po dims at partitions 0-63, den at 64
        # odd  heads use lhsT [:,64:192]-> po den at partition 0, dims at 64-127
        v_ext = [attnp.tile([128, 192], bf16, name=f"vext{kt}") for kt in range(KT)]
        for kt in range(KT):
            nc.vector.memset(v_ext[kt][:, 64:128], 0.0)
            nc.vector.memset(v_ext[kt][:, 64:65], 1.0)
        for r in range(4):
            vT_sb = work.tile([64, R], bf16, tag="vTs", bufs=1, name="vT_sb")
            nc.sync.dma_start(
                out=vT_sb, in_=kvT_all[r * 128 + 64 : (r + 1) * 128, :]
            )
            for cc in range(4):
                kt = r * 4 + cc
                ptv = ps_start.tile([128, 128], bf16, tag="tp", bufs=2, name="ptv")
                nc.tensor.matmul(
                    ptv[:, 0:64],
                    vT_sb[:, cc * 128 : (cc + 1) * 128],
                    ident[0:64, 0:64],
                    is_transpose=True,
                )
                nc.vector.tensor_copy(out=v_ext[kt][:, 0:64], in_=ptv[:, 0:64])
                nc.vector.tensor_copy(out=v_ext[kt][:, 128:192], in_=ptv[:, 0:64])

        den16 = attnp.tile([16, R], f32, name="den16")
        rcpz = attnp.tile([128, R], bf16, name="rcpz")
        nc.vector.memset(rcpz, 0.0)

        cm_ps_start.__exit__(None, None, None)

        # attention-era PSUM: 3 x [128,1024] (6 banks) + 2 x po (2 banks)
        cm_ps_attn = tc.tile_pool(name="ps_attn", bufs=1, space="PSUM")
        ps_attn = cm_ps_attn.__enter__()

        # out2[hp2]: attention output stacked by head pair (head 2*hp2 on
        # partitions 0-63, head 2*hp2+1 on 64-127), raw (pre-normalization).
        out2 = [midp.tile([128, R], bf16, name=f"out2_{hp}") for hp in range(H // 2)]

        # ---------------- phase 4: attention ----------------
        for h in range(H):
            mt, even = h // 2, (h % 2) == 0
            kTs = kT_lo if even else kT_hi
            po = ps_attn.tile([128, 512], f32, tag="po", bufs=2, name="po")
            for c in range(KT // 2):
                psc = ps_attn.tile([128, 1024], f32, tag="ps2", bufs=3, name="psc")
                for half in range(2):
                    kt = 2 * c + half
                    nc.tensor.matmul(
                        psc[:, half * 512 : (half + 1) * 512],
                        kTs[:, kt * 128 : (kt + 1) * 128],
                        qT[mt],
                        start=True,
                        stop=True,
                    )
                pr = attnp.tile([128, 1024], bf16, tag="pr", bufs=4, name="pr")
                nc.scalar.activation(out=pr, in_=psc, func=AF.Exp)
                for half in range(2):
                    kt = 2 * c + half
                    lhs = v_ext[kt][:, 0:65] if even else v_ext[kt][:, 64:192]
                    outsl = po[0:65, :] if even else po[0:128, :]
                    nc.tensor.matmul(
                        outsl,
                        lhs,
                        pr[:, half * 512 : (half + 1) * 512],
                        start=(c == 0 and half == 0),
                        stop=(c == KT // 2 - 1 and half == 1),
                    )
            # stage the denominator row to SBUF (same-partition DVE copy),
            # then cross-partition SBUF->SBUF DMA into the den16 gather tile
            stg = work.tile([128, 512], f32, tag="dstg", name="dstg")
            if even:
                nc.vector.tensor_copy(out=stg[64:65, :], in_=po[64:65, :])
                nc.gpsimd.dma_start(out=den16[h : h + 1, :], in_=stg[64:65, :])
                nc.vector.tensor_copy(out=out2[mt][0:64, :], in_=po[0:64, :])
            else:
                nc.vector.tensor_copy(out=stg[0:1, :], in_=po[0:1, :])
                nc.gpsimd.dma_start(out=den16[h : h + 1, :], in_=stg[0:1, :])
                nc.vector.tensor_copy(out=out2[mt][64:128, :], in_=po[64:128, :])

        cm_ps_attn.__exit__(None, None, None)

        cm_ps_p5 = tc.tile_pool(name="ps_p5", bufs=1, space="PSUM")
        ps_p5 = cm_ps_p5.__enter__()

        # ---------------- phase 5: normalize + out proj + residual -> x1 ----------------
        with nc.allow_low_precision(reason="bf16 softmax denom broadcast"):
            nc.vector.reciprocal(out=rcpz[0:16, :], in_=den16)
        for hp2 in range(H // 2):
            bcrt = ps_p5.tile([128, 512], f32, tag="bcr", bufs=2, name="bcrt")
            nc.tensor.matmul(
                bcrt, sel2[:, hp2 * 128 : (hp2 + 1) * 128], rcpz, start=True, stop=True
            )
            nc.vector.tensor_mul(out=out2[hp2], in0=out2[hp2], in1=bcrt)

        x1 = [persist.tile([128, F], f32, name=f"x1_{rt}") for rt in range(RB)]
        for rt in range(RB):
            for fh in range(2):
                px = ps_p5.tile([128, 512], f32, tag="px", bufs=3, name="px")
                for hp2 in range(H // 2):
                    nc.tensor.matmul(
                        px,
                        out2[hp2][:, rt * 128 : (rt + 1) * 128],
                        wo2sb[hp2][:, fh * 512 : (fh + 1) * 512],
                        start=(hp2 == 0),
                        stop=(hp2 == H // 2 - 1),
                    )
                sl = slice(fh * 512, (fh + 1) * 512)
                nc.vector.tensor_add(out=x1[rt][:, sl], in0=px, in1=h_res[rt][:, sl])
                nc.vector.tensor_add(
                    out=x1[rt][:, sl], in0=x1[rt][:, sl], in1=wob_bc[:, sl]
                )

        cm_ps_p5.__exit__(None, None, None)
        cm_mid.__exit__(None, None, None)
        cm_attn.__exit__(None, None, None)
        cm_hera.__exit__(None, None, None)

        cm_ps_p6 = tc.tile_pool(name="ps_p6", bufs=1, space="PSUM")
        ps_p6 = cm_ps_p6.__enter__()

        cm_mlps = tc.tile_pool(name="mlps", bufs=1)
        mlps = cm_mlps.__enter__()

        # ---------------- phase 6: adaLN-2 + transpose ----------------
        # LN core on DVE; Wf/Bf applied as per-partition scale/bias in the
        # PSUM->SBUF transpose copies. b2 is folded into x1 right after its
        # LN stats are consumed (y = mlp2 + (x1 + b2)).
        h2T = [mlps.tile([128, R], bf16, name=f"h2T{ft}") for ft in range(FT)]
        for rt in range(RB):
            mv3, rstd3 = ln_stats(x1[rt])
            xn_bf = work.tile([128, F], bf16, tag="xn", name="xn2_bf")
            nc.vector.tensor_scalar(
                out=xn_bf,
                in0=x1[rt],
                scalar1=mv3[:, 0:1],
                scalar2=rstd3,
                op0=OP.subtract,
                op1=OP.mult,
            )
            nc.vector.tensor_add(out=x1[rt], in0=x1[rt], in1=b2_bc)
            for ft in range(FT):
                pt = ps_p6.tile([128, 128], bf16, tag="tp2", bufs=2, name="pt2")
                nc.tensor.transpose(pt, xn_bf[:, ft * 128 : (ft + 1) * 128], ident)
                nc.scalar.activation(
                    out=h2T[ft][:, rt * 128 : (rt + 1) * 128],
                    in_=pt,
                    func=AF.Identity,
                    bias=BfC[:, ft : ft + 1],
                    scale=WfC[:, ft : ft + 1],
                )

        cm_ps_p6.__exit__(None, None, None)

        cm_ps_mlp = tc.tile_pool(name="ps_mlp", bufs=1, space="PSUM")
        ps_mlp = cm_ps_mlp.__enter__()

        # ---------------- phase 7: mlp1 + gelu ----------------
        g1T = [mlps.tile([128, R], bf16, name=f"g1T{mt}") for mt in range(MFT)]
        for mt in range(MFT):
            w1blk = wstr.tile([128, FT * 128], bf16, tag="w1b", bufs=3, name="w1blk")
            nc.sync.dma_start(out=w1blk, in_=w1_d[mt])
            pg = ps_mlp.tile([128, 512], f32, tag="pg", bufs=3, name="pg")
            for kt in range(FT):
                nc.tensor.matmul(
                    pg,
                    w1blk[:, kt * 128 : (kt + 1) * 128],
                    h2T[kt],
                    start=(kt == 0),
                    stop=(kt == FT - 1),
                )
            nc.scalar.activation(
                out=g1T[mt],
                in_=pg,
                func=AF.Gelu,
                bias=b1_sb[:, mt : mt + 1],
                scale=1.0,
            )

        # ---------------- phase 8: mlp2 + residual -> y ----------------
        # mlp2's kt-th accumulation step only needs g1T[kt], so the fh=0
        # column sweep pipelines with mlp1 on the PE.
        for fh in range(2):
            pf = {}
            for rt in range(RB):
                pf[rt] = ps_mlp.tile(
                    [128, 512], f32, tag=f"pf{rt}", bufs=1, name=f"pf{rt}"
                )
            for kt in range(MFT):
                w2c = wstr.tile([128, 512], bf16, tag="w2c", bufs=3, name="w2c")
                nc.gpsimd.dma_start(
                    out=w2c, in_=w2_d[kt * 128 : (kt + 1) * 128, fh * 512 : (fh + 1) * 512]
                )
                for rt in range(RB):
                    nc.tensor.matmul(
                        pf[rt],
                        g1T[kt][:, rt * 128 : (rt + 1) * 128],
                        w2c,
                        start=(kt == 0),
                        stop=(kt == MFT - 1),
                    )
            for rt in range(RB):
                sl = slice(fh * 512, (fh + 1) * 512)
                yh = work.tile([128, 512], f32, tag="yh", bufs=2, name="yh")
                nc.vector.tensor_add(out=yh, in0=pf[rt], in1=x1[rt][:, sl])
                nc.sync.dma_start(out=y_d[rt * 128 : (rt + 1) * 128, sl], in_=yh)

        cm_ps_mlp.__exit__(None, None, None)
        cm_mlps.__exit__(None, None, None)

    nc.compile()
    return nc


def _prep_in_maps(inputs):
    f32 = np.float32
    wmod = np.concatenate(
        [inputs["amod_gw"], inputs["amod_bw"], inputs["fmod_gw"], inputs["fmod_bw"]],
        axis=1,
    ).astype(BF16)
    modb = np.concatenate(
        [inputs["amod_gb"], inputs["amod_bb"], inputs["fmod_gb"], inputs["fmod_bb"]]
    ).astype(f32)
    lnvec = np.stack(
        [
            inputs["amod_nw"],
            inputs["amod_nb"],
            inputs["attn_nw"],
            inputs["attn_nb"],
            inputs["fmod_nw"],
            inputs["fmod_nb"],
        ]
    ).astype(f32)
    wq_t = np.ascontiguousarray(
        np.asarray(inputs["wq"]).astype(BF16).reshape(FT, 128, MT, 128)
        .transpose(2, 1, 0, 3).reshape(MT, 128, FT * 128)
    )
    w1_t = np.ascontiguousarray(
        np.asarray(inputs["w1"]).astype(BF16).reshape(FT, 128, MFT, 128)
        .transpose(2, 1, 0, 3).reshape(MFT, 128, FT * 128)
    )
    sel2 = np.zeros((128, H * 64), BF16)
    for hp2 in range(H // 2):
        sel2[2 * hp2, hp2 * 128 : hp2 * 128 + 64] = 1
        sel2[2 * hp2 + 1, hp2 * 128 + 64 : hp2 * 128 + 128] = 1
    shared = dict(
        sel2=sel2,
        wmod=wmod,
        modb=modb,
        lnvec=lnvec,
        wq=wq_t,
        wkv=np.asarray(inputs["wkv"]).astype(BF16),
        wo=np.asarray(inputs["wo"]).astype(BF16),
        wo_bias=np.asarray(inputs["wo_b"]).astype(f32).reshape(1, F),
        w1=w1_t,
        b1=np.asarray(inputs["b1"]).astype(f32),
        w2=np.asarray(inputs["w2"]).astype(BF16),
        b2=np.asarray(inputs["b2"]).astype(f32).reshape(1, F),
    )
    x = np.asarray(inputs["x"]).astype(f32)
    cond = np.asarray(inputs["cond"]).astype(BF16)
    in_maps = []
    for c in range(NCORES):
        b, r0 = c // 4, (c % 4) * R
        m = dict(shared)
        m["x"] = np.ascontiguousarray(x[b, r0 : r0 + R, :])
        m["cond"] = np.ascontiguousarray(cond[b])
        in_maps.append(m)
    return in_maps


def _run(inputs, trace=False):
    from concourse.bass_utils import run_bass_kernel_spmd

    if "nc" not in _CACHE:
        _CACHE["nc"] = _build_nc()
    nc = _CACHE["nc"]
    in_maps = _prep_in_maps(inputs)
    res = run_bass_kernel_spmd(
        nc, in_maps, core_ids=list(range(NCORES)), trace=trace
    )
    y = np.empty((B, T, F), np.float32)
    for c in range(NCORES):
        b, r0 = c // 4, (c % 4) * R
        y[b, r0 : r0 + R, :] = res.results[c]["y"]
    return y, res


def kernel(**inputs) -> np.ndarray:
    y, _ = _run(inputs, trace=False)
    return y


if __name__ == "__main__":
    _build_nc()
    print("build OK")


# revision 12
# speedup vs baseline: 1.7225x; 1.0221x over previous
"""DiT block kernel for 8x Trainium2 NeuronCores (Bass/Tile).

Sharding: row-parallel over the flattened (B,T)=4096 rows; 512 rows/core.
Cores 0-3 handle batch 0, cores 4-7 batch 1. MQA K/V is computed per-shard
and AllGather'd within each 4-core batch group. Weights are replicated and
cast to bf16; LN/residual math stays fp32.

v2 structure notes (driven by trace analysis of v1):
  - ALL attention matmuls run in the PE's (128,128) tile mode: QK^T uses
    zero-padded kT tiles (kT_lo has k on partitions 0-63 and zeros above,
    kT_hi the reverse) so the contraction is always 128-wide.  v1 alternated
    (64,128) MM1s with (128,128) PV matmuls instruction-by-instruction and
    every matmul paid a PE mode-switch drain (~2.5x slowdown).
  - exp() is applied to [128,1024] PSUM chunks (2 key tiles at once) to
    amortize the ~370ns ACT SBUF/PSUM access latency per instruction.
  - softmax denominators are gathered (via tiny PSUM->SBUF DMAs) into one
    [16,512] tile and inverted with ONE DVE reciprocal (v1: 16 calls at
    2.4us each), then broadcast per head-pair with a single (128,128)-mode
    select-matmul.
  - attention outputs land directly in head-pair-stacked tiles [128,R]
    (even head on partitions 0-63, odd on 64-127, via a dual-layout v_ext)
    so the out-projection contracts 128 partitions per matmul (v1: 64).
  - the adaLN-2 / attn-LN scale+bias are folded into the PSUM->SBUF copies
    of the transposes (per-partition scale/bias APs on ACT).
  - mod vectors, LN chains, DMA queue placement all restructured so the
    K/V AllGather is issued ~30us into the kernel instead of ~125us.
"""

import sys

sys.path.insert(0, "/opt/trn_rl_repo")

import numpy as np
import ml_dtypes

BF16 = ml_dtypes.bfloat16

B, T, F, H, D, M, C = 2, 2048, 1024, 16, 64, 4, 512
NCORES = 8
R = (B * T) // NCORES  # 512 rows per core
RB = R // 128  # 4 row blocks
FT = F // 128  # 8 feature tiles
MT = (H * D) // 128  # 8 head-pair tiles
MFT = (M * F) // 128  # 32 mlp hidden tiles
KT = T // 128  # 16 key tiles
EPS = 1e-5

_CACHE = {}


def _build_nc():
    import concourse.bass as bass
    import concourse.tile as tile
    from concourse import bacc, mybir
    from concourse.masks import make_identity
    from contextlib import ExitStack

    f32 = mybir.dt.float32
    f16 = mybir.dt.float16
    bf16 = mybir.dt.bfloat16
    AF = mybir.ActivationFunctionType
    OP = mybir.AluOpType

    nc = bacc.Bacc(
        "TRN2",
        target_bir_lowering=False,
        debug=False,
        enable_asserts=False,
        num_devices=NCORES,
    )

    def dram(name, shape, dt, **kw):
        return nc.dram_tensor(name, shape, dt, **kw).ap()

    x_d = dram("x", [R, F], f32, kind="ExternalInput")
    cond_d = dram("cond", [C], bf16, kind="ExternalInput")
    wmod_d = dram("wmod", [C, 4 * F], bf16, kind="ExternalInput")
    modb_d = dram("modb", [4 * F], f32, kind="ExternalInput")
    lnv_d = dram("lnvec", [6, F], f32, kind="ExternalInput")
    wq_d = dram("wq", [MT, 128, FT * 128], bf16, kind="ExternalInput")
    wkv_d = dram("wkv", [128, FT, 2 * D], bf16, kind="ExternalInput")
    wo_d = dram("wo", [H * D, F], bf16, kind="ExternalInput")
    wob_d = dram("wo_bias", [1, F], f32, kind="ExternalInput")
    w1_d = dram("w1", [MFT, 128, FT * 128], bf16, kind="ExternalInput")
    b1_d = dram("b1", [M * F], f32, kind="ExternalInput")
    w2_d = dram("w2", [M * F, F], bf16, kind="ExternalInput")
    b2_d = dram("b2", [1, F], f32, kind="ExternalInput")
    sel2_d = dram("sel2", [128, H * 64], bf16, kind="ExternalInput")
    y_d = dram("y", [R, F], f32, kind="ExternalOutput")

    groups = [[0, 1, 2, 3], [4, 5, 6, 7]]

    def bcast_row(ap_row):
        # [1, n] DRAM AP -> partition-broadcast [128, n]
        return bass.AP(
            tensor=ap_row.tensor,
            offset=ap_row.offset,
            ap=[[0, 128]] + list(ap_row.ap[-1:]),
        )

    def row_cols(ap2d, r):
        # row r of a [*, F] DRAM AP, viewed as [128, FT] columns:
        # out[p, t] = row[t*128 + p]
        row = bass.AP(
            tensor=ap2d.tensor,
            offset=ap2d.offset + r * F,
            ap=[[1, F]],
        )
        return row.rearrange("(t p) -> p t", p=128)

    with tile.TileContext(nc) as tc, ExitStack() as ctx:
        consts = ctx.enter_context(tc.tile_pool(name="consts", bufs=1))
        work = ctx.enter_context(tc.tile_pool(name="work", bufs=2))
        persist = ctx.enter_context(tc.tile_pool(name="persist", bufs=1))
        wstr = ctx.enter_context(tc.tile_pool(name="wstr", bufs=3))
        dramp = ctx.enter_context(tc.tile_pool(name="dramp", bufs=1, space="DRAM"))

        # ---------------- constants ----------------
        ident = consts.tile([128, 128], bf16, name="ident")
        make_identity(nc, ident)
        epst = consts.tile([128, 1], f32, name="epst")
        nc.vector.memset(epst, EPS)

        cond_sb = consts.tile([128, 4], bf16, name="cond_sb")
        nc.sync.dma_start(out=cond_sb, in_=cond_d.rearrange("(a p) -> p a", p=128))
        b1_sb = consts.tile([128, MFT], f32, name="b1_sb")
        nc.gpsimd.dma_start(out=b1_sb, in_=b1_d.rearrange("(mt p) -> p mt", p=128))
        wkv_sb = consts.tile([128, FT, 2 * D], bf16, name="wkv_sb")
        nc.gpsimd.dma_start(out=wkv_sb, in_=wkv_d)
        # attn-LN weight/bias in column layout (per-partition scale/bias of
        # the transpose copies)
        anwT = consts.tile([128, FT], f32, name="anwT")
        nc.gpsimd.dma_start(out=anwT, in_=row_cols(lnv_d, 2))
        anbT = consts.tile([128, FT], f32, name="anbT")
        nc.gpsimd.dma_start(out=anbT, in_=row_cols(lnv_d, 3))
        wob_bc = consts.tile([128, F], f32, name="wob_bc")
        nc.gpsimd.dma_start(out=wob_bc, in_=bcast_row(wob_d[0:1, :]))
        b2_bc = consts.tile([128, F], f32, name="b2_bc")
        nc.gpsimd.dma_start(out=b2_bc, in_=bcast_row(b2_d[0:1, :]))

        # head-pair select matrix for the softmax-denominator broadcast
        # (host-built: block hp2 selects den row 2*hp2 into output partitions
        # 0-63 and row 2*hp2+1 into partitions 64-127)
        sel2 = consts.tile([128, H * 64], bf16, name="sel2")
        nc.gpsimd.dma_start(out=sel2, in_=sel2_d)

        # phase-scoped SBUF pools, opened in LIFO-compatible nesting order
        # (innermost closes first): hera > attnp > midp > [psum scopes]
        cm_hera = tc.tile_pool(name="hera", bufs=1)  # hT
        hera = cm_hera.__enter__()
        cm_attn = tc.tile_pool(name="attnp", bufs=1)  # attention-era tiles
        attnp = cm_attn.__enter__()
        cm_mid = tc.tile_pool(name="midp", bufs=1)  # h_res/out2/wo2 era
        midp = cm_mid.__enter__()

        # PSUM pool for the startup phases (mod / transposes / kv / q / v_ext)
        cm_ps_start = tc.tile_pool(name="ps_start", bufs=1, space="PSUM")
        ps_start = cm_ps_start.__enter__()

        # ---------------- phase 0: modulation vectors ----------------
        # modv = cond @ [gA | bA | gF | bF] + modb, kept in a row-chunked
        # [8, 512] layout (chunk nb on partition nb) so the finalize math
        # runs on multiple DVE lanes and SBUF address-space cost stays tiny.
        # Wa/Ba are then round-tripped through DRAM into partition-broadcast
        # [128,F] tiles; Wf/Bf into column [128,FT] tiles.
        cm_modtmp = tc.tile_pool(name="modtmp", bufs=1)
        modtmp = cm_modtmp.__enter__()

        def row_chunked(ap2d, r, p):
            # row r of a [*, F-or-4F] DRAM AP as [p, n/p] (chunk c on lane c)
            n = ap2d.ap[-1][1]
            row = bass.AP(
                tensor=ap2d.tensor,
                offset=ap2d.offset + r * n,
                ap=[[1, n]],
            )
            return row.rearrange("(p j) -> p j", p=p)

        lnr8 = {}
        for r in (0, 1, 4, 5):  # amod_nw/nb, fmod_nw/nb as [2, 512]
            lnr8[r] = modtmp.tile([2, 512], f32, name=f"lnr8_{r}")
            nc.sync.dma_start(out=lnr8[r], in_=row_chunked(lnv_d, r, 2))
        # four [2,512] vectors (gA, bA, gF, bF), each at base partition 0
        # (compute engines require base partition in {0,32,64,96})
        mvec = [modtmp.tile([2, 512], f32, name=f"mvec{v}") for v in range(4)]
        mbias = [modtmp.tile([2, 512], f32, name=f"mbias{v}") for v in range(4)]
        for v in range(4):
            nc.sync.dma_start(
                out=mbias[v],
                in_=row_chunked(modb_d.rearrange("(a f) -> a f", a=4), v, 2),
            )
        tmp2 = modtmp.tile([2, 512], f32, name="tmp2")
        mod_dram = dramp.tile([4, F], f32, name="mod_dram")

        def mod_half(nbs, gi, bi, nw_r, nb_r):
            for nb in nbs:
                pm = ps_start.tile([128, 512], f32, tag="sp", bufs=3, name="pm")
                for ch in range(4):
                    wm = modtmp.tile(
                        [128, 512], bf16, tag="wm", bufs=4, name="wm"
                    )
                    eng = nc.sync if ch % 2 == 0 else nc.gpsimd
                    eng.dma_start(
                        out=wm,
                        in_=wmod_d[
                            ch * 128 : (ch + 1) * 128,
                            nb * 512 : (nb + 1) * 512,
                        ],
                    )
                    nc.tensor.matmul(
                        pm[0:1, :],
                        cond_sb[:, ch : ch + 1],
                        wm,
                        start=(ch == 0),
                        stop=(ch == 3),
                    )
                pmrow = modtmp.tile([1, 512], f32, tag="pmrow", bufs=2, name="pmrow")
                nc.vector.tensor_copy(out=pmrow, in_=pm[0:1, :])
                # relocate chunk nb to lane nb%2 of vector nb//2
                nc.sync.dma_start(
                    out=mvec[nb // 2][nb % 2 : nb % 2 + 1, :], in_=pmrow
                )
            for v in (gi, bi):
                nc.vector.tensor_add(out=mvec[v], in0=mvec[v], in1=mbias[v])
            # finalize: g := nw*(1+g), b := nb*(1+g) + b
            nc.scalar.add(out=mvec[gi], in_=mvec[gi], add=1.0)
            nc.vector.tensor_mul(out=tmp2, in0=mvec[gi], in1=lnr8[nb_r])
            nc.vector.tensor_add(out=mvec[bi], in0=tmp2, in1=mvec[bi])
            nc.vector.tensor_mul(out=mvec[gi], in0=mvec[gi], in1=lnr8[nw_r])
            for v in (gi, bi):
                nc.sync.dma_start(out=row_chunked(mod_dram, v, 2), in_=mvec[v])

        # amod half first: Wa_bc/Ba_bc gate all of phase 1
        mod_half((0, 1, 2, 3), 0, 1, 0, 1)
        bc = {}
        for v, nm in ((0, "Wa_bc"), (1, "Ba_bc")):
            bt = consts.tile([128, F], f32, name=nm)
            nc.sync.dma_start(out=bt, in_=bcast_row(mod_dram[v : v + 1, :]))
            bc[nm] = bt
        # fmod half (consumed only at phase 6)
        mod_half((4, 5, 6, 7), 2, 3, 4, 5)
        WfC = consts.tile([128, FT], f32, name="WfC")
        nc.sync.dma_start(out=WfC, in_=row_cols(mod_dram, 2))
        BfC = consts.tile([128, FT], f32, name="BfC")
        nc.sync.dma_start(out=BfC, in_=row_cols(mod_dram, 3))

        cm_modtmp.__exit__(None, None, None)

        # ---------------- LN stats helper ----------------
        def ln_stats(src):
            stats = work.tile([128, 2, 6], f32, tag="stats", name="stats")
            for sg in range(2):
                nc.vector.bn_stats(
                    out=stats[:, sg, :], in_=src[:, sg * 512 : (sg + 1) * 512]
                )
            mv = work.tile([128, 2], f32, tag="mv", name="mv")
            nc.vector.bn_aggr(out=mv, in_=stats)
            rstd = work.tile([128, 1], f32, tag="rstd", name="rstd")
            nc.scalar.activation(
                out=rstd, in_=mv[:, 1:2], func=AF.Sqrt, bias=epst, scale=1.0
            )
            nc.vector.reciprocal(out=rstd, in_=rstd)
            return mv, rstd

        hT = [hera.tile([128, R], bf16, name=f"hT{ft}") for ft in range(FT)]
        h_res = [midp.tile([128, F], bf16, name=f"h{rb}") for rb in range(RB)]

        # ---------------- phase 1: adaLN-1 + attn-LN + transpose ----------------
        for rb in range(RB):
            x_rb = work.tile([128, F], f32, tag="x", name="x_rb")
            nc.scalar.dma_start(out=x_rb, in_=x_d[rb * 128 : (rb + 1) * 128, :])
            mv1, rstd1 = ln_stats(x_rb)
            nc.vector.scalar_tensor_tensor(
                out=x_rb,
                in0=x_rb,
                scalar=mv1[:, 0:1],
                in1=bc["Wa_bc"],
                op0=OP.subtract,
                op1=OP.mult,
            )
            nc.vector.scalar_tensor_tensor(
                out=h_res[rb],
                in0=x_rb,
                scalar=rstd1,
                in1=bc["Ba_bc"],
                op0=OP.mult,
                op1=OP.add,
            )
            mv2, rstd2 = ln_stats(h_res[rb])
            xn_bf = work.tile([128, F], bf16, tag="xn", name="xn_bf")
            nc.vector.tensor_scalar(
                out=xn_bf,
                in0=h_res[rb],
                scalar1=mv2[:, 0:1],
                scalar2=rstd2,
                op0=OP.subtract,
                op1=OP.mult,
            )
            for ft in range(FT):
                pt = ps_start.tile([128, 128], bf16, tag="tp", bufs=2, name="pt")
                nc.tensor.transpose(pt, xn_bf[:, ft * 128 : (ft + 1) * 128], ident)
                nc.scalar.activation(
                    out=hT[ft][:, rb * 128 : (rb + 1) * 128],
                    in_=pt,
                    func=AF.Identity,
                    bias=anbT[:, ft : ft + 1],
                    scale=anwT[:, ft : ft + 1],
                )

        # ---------------- phase 2: kv first (AllGather ASAP), then q ----------------
        pkv = ps_start.tile([128, 512], f32, tag="sp", bufs=3, name="pkv")
        for kt in range(FT):
            nc.tensor.matmul(
                pkv, wkv_sb[:, kt, :], hT[kt], start=(kt == 0), stop=(kt == FT - 1)
            )
        kvT_sb = work.tile([128, R], bf16, tag="kvT", bufs=1, name="kvT_sb")
        nc.scalar.activation(out=kvT_sb, in_=pkv, func=AF.Copy)

        kvT_bounce = dramp.tile([2 * D, R], bf16, name="kvT_bounce")
        kvT_all = dramp.tile([4 * 2 * D, R], bf16, name="kvT_all")
        nc.sync.dma_start(out=kvT_bounce, in_=kvT_sb)
        nc.gpsimd.collective_compute(
            "AllGather",
            OP.bypass,
            replica_groups=groups,
            ins=[kvT_bounce[:, :]],
            outs=[kvT_all[:, :]],
        )

        # q projection fills the AllGather wait
        qT = [attnp.tile([128, R], bf16, name=f"qT{mt}") for mt in range(MT)]
        for mt in range(MT):
            wqblk = wstr.tile([128, FT * 128], bf16, tag="wqb", bufs=3, name="wqblk")
            nc.sync.dma_start(out=wqblk, in_=wq_d[mt])
            pq = ps_start.tile([128, 512], f32, tag="sp", bufs=3, name="pq")
            for kt in range(FT):
                nc.tensor.matmul(
                    pq,
                    wqblk[:, kt * 128 : (kt + 1) * 128],
                    hT[kt],
                    start=(kt == 0),
                    stop=(kt == FT - 1),
                )
            # fold the attention 1/sqrt(D)=0.125 scale into q
            nc.scalar.activation(out=qT[mt], in_=pq, func=AF.Copy, scale=0.125)

        # fold the out-proj bias into the residual while the AllGather is in
        # flight (x1 = px + (h_res + wo_b))
        for rb in range(RB):
            nc.vector.tensor_add(out=h_res[rb], in0=h_res[rb], in1=wob_bc)

        # preload exp's ACT table set during the AllGather window
        warm = work.tile([1, 1], f32, tag="warm", bufs=1, name="warm")
        nc.scalar.activation(out=warm, in_=epst[0:1, 0:1], func=AF.Exp)

        # prefetch wo (consumed in phase 5)
        wo2sb = [midp.tile([128, F], bf16, name=f"wo2_{hp}") for hp in range(MT)]
        for hp in range(MT):
            nc.gpsimd.dma_start(out=wo2sb[hp], in_=wo_d[hp * 128 : (hp + 1) * 128, :])

        # ---------------- phase 3: kT / v_ext assembly ----------------
        # kT_lo: k on partitions 0-63, zeros above  -> even heads (q on 0-63)
        # kT_hi: k on partitions 64-127, zeros below -> odd heads
        # Both give contraction-128 matmuls: no PE tile-mode switching.
        # v_ext dual layout [128, 192]:
        #   cols 0-63 = v dims, col 64 = ones, cols 65-127 = 0, cols 128-191 = v dims
        # even heads use lhsT [:,0:65]  -> po dims at partitions 0-63, den at 64
        # odd  heads use lhsT [:,64:192]-> po den at partition 0, dims at 64-127
        v_ext = [attnp.tile([128, 192], bf16, name=f"vext{kt}") for kt in range(KT)]
        for kt in range(KT):
            nc.vector.memset(v_ext[kt][:, 64:128], 0.0)
            nc.vector.memset(v_ext[kt][:, 64:65], 1.0)
        for r in range(4):
            vT_sb = work.tile([64, R], bf16, tag="vTs", bufs=1, name="vT_sb")
            nc.sync.dma_start(
                out=vT_sb, in_=kvT_all[r * 128 + 64 : (r + 1) * 128, :]
            )
            for cc in range(4):
                kt = r * 4 + cc
                ptv = ps_start.tile([128, 128], bf16, tag="tp", bufs=2, name="ptv")
                nc.tensor.matmul(
                    ptv[:, 0:64],
                    vT_sb[:, cc * 128 : (cc + 1) * 128],
                    ident[0:64, 0:64],
                    is_transpose=True,
                )
                nc.vector.tensor_copy(out=v_ext[kt][:, 0:64], in_=ptv[:, 0:64])
                nc.vector.tensor_copy(out=v_ext[kt][:, 128:192], in_=ptv[:, 0:64])

        kT_lo = attnp.tile([128, T], bf16, name="kT_lo")
        kT_hi = attnp.tile([128, T], bf16, name="kT_hi")
        nc.vector.memset(kT_lo[64:128, :], 0.0)
        nc.vector.memset(kT_hi[0:64, :], 0.0)
        for r in range(4):
            nc.sync.dma_start(
                out=kT_lo[0:64, r * R : (r + 1) * R],
                in_=kvT_all[r * 128 : r * 128 + 64, :],
            )
            nc.gpsimd.dma_start(
                out=kT_hi[64:128, r * R : (r + 1) * R],
                in_=kvT_all[r * 128 : r * 128 + 64, :],
            )
        den16 = attnp.tile([16, R], f32, name="den16")
        rcpz = attnp.tile([128, R], bf16, name="rcpz")
        nc.vector.memset(rcpz, 0.0)

        cm_ps_start.__exit__(None, None, None)

        # attention-era PSUM: 3 x [128,1024] (6 banks) + 2 x po (2 banks)
        cm_ps_attn = tc.tile_pool(name="ps_attn", bufs=1, space="PSUM")
        ps_attn = cm_ps_attn.__enter__()

        # out2[hp2]: attention output stacked by head pair (head 2*hp2 on
        # partitions 0-63, head 2*hp2+1 on 64-127), raw (pre-normalization).
        out2 = [midp.tile([128, R], bf16, name=f"out2_{hp}") for hp in range(H // 2)]

        # ---------------- phase 4: attention ----------------
        for h in range(H):
            mt, even = h // 2, (h % 2) == 0
            kTs = kT_lo if even else kT_hi
            po = ps_attn.tile([128, 512], f32, tag="po", bufs=2, name="po")
            for c in range(KT // 2):
                psc = ps_attn.tile([128, 1024], f32, tag="ps2", bufs=3, name="psc")
                for half in range(2):
                    kt = 2 * c + half
                    nc.tensor.matmul(
                        psc[:, half * 512 : (half + 1) * 512],
                        kTs[:, kt * 128 : (kt + 1) * 128],
                        qT[mt],
                        start=True,
                        stop=True,
                    )
                pr = attnp.tile([128, 1024], bf16, tag="pr", bufs=4, name="pr")
                nc.scalar.activation(out=pr, in_=psc, func=AF.Exp)
                for half in range(2):
                    kt = 2 * c + half
                    lhs = v_ext[kt][:, 0:65] if even else v_ext[kt][:, 64:192]
                    outsl = po[0:65, :] if even else po[0:128, :]
                    nc.tensor.matmul(
                        outsl,
                        lhs,
                        pr[:, half * 512 : (half + 1) * 512],
                        start=(c == 0 and half == 0),
                        stop=(c == KT // 2 - 1 and half == 1),
                    )
            # stage the denominator row to SBUF (same-partition DVE copy),
            # then cross-partition SBUF->SBUF DMA into the den16 gather tile
            stg = work.tile([128, 512], f32, tag="dstg", name="dstg")
            if even:
                nc.vector.tensor_copy(out=stg[64:65, :], in_=po[64:65, :])
                nc.gpsimd.dma_start(out=den16[h : h + 1, :], in_=stg[64:65, :])
                nc.vector.tensor_copy(out=out2[mt][0:64, :], in_=po[0:64, :])
            else:
                nc.vector.tensor_copy(out=stg[0:1, :], in_=po[0:1, :])
                nc.gpsimd.dma_start(out=den16[h : h + 1, :], in_=stg[0:1, :])
                nc.vector.tensor_copy(out=out2[mt][64:128, :], in_=po[64:128, :])

        cm_ps_attn.__exit__(None, None, None)

        cm_ps_p5 = tc.tile_pool(name="ps_p5", bufs=1, space="PSUM")
        ps_p5 = cm_ps_p5.__enter__()

        # ---------------- phase 5: normalize + out proj + residual -> x1 ----------------
        with nc.allow_low_precision(reason="bf16 softmax denom broadcast"):
            nc.vector.reciprocal(out=rcpz[0:16, :], in_=den16)
        for hp2 in range(H // 2):
            bcrt = ps_p5.tile([128, 512], f32, tag="bcr", bufs=2, name="bcrt")
            nc.tensor.matmul(
                bcrt, sel2[:, hp2 * 128 : (hp2 + 1) * 128], rcpz, start=True, stop=True
            )
            nc.vector.tensor_mul(out=out2[hp2], in0=out2[hp2], in1=bcrt)

        x1 = [persist.tile([128, F], f32, name=f"x1_{rt}") for rt in range(RB)]
        for rt in range(RB):
            for fh in range(2):
                px = ps_p5.tile([128, 512], f32, tag="px", bufs=3, name="px")
                for hp2 in range(H // 2):
                    nc.tensor.matmul(
                        px,
                        out2[hp2][:, rt * 128 : (rt + 1) * 128],
                        wo2sb[hp2][:, fh * 512 : (fh + 1) * 512],
                        start=(hp2 == 0),
                        stop=(hp2 == H // 2 - 1),
                    )
                sl = slice(fh * 512, (fh + 1) * 512)
                nc.vector.tensor_add(out=x1[rt][:, sl], in0=px, in1=h_res[rt][:, sl])

        cm_ps_p5.__exit__(None, None, None)
        cm_mid.__exit__(None, None, None)
        cm_attn.__exit__(None, None, None)
        cm_hera.__exit__(None, None, None)

        cm_ps_p6 = tc.tile_pool(name="ps_p6", bufs=1, space="PSUM")
        ps_p6 = cm_ps_p6.__enter__()

        cm_mlps = tc.tile_pool(name="mlps", bufs=1)
        mlps = cm_mlps.__enter__()

        # ---------------- phase 6: adaLN-2 + transpose ----------------
        # LN core on DVE; Wf/Bf applied as per-partition scale/bias in the
        # PSUM->SBUF transpose copies. b2 is folded into x1 right after its
        # LN stats are consumed (y = mlp2 + (x1 + b2)).
        h2T = [mlps.tile([128, R], bf16, name=f"h2T{ft}") for ft in range(FT)]
        for rt in range(RB):
            mv3, rstd3 = ln_stats(x1[rt])
            xn_bf = work.tile([128, F], bf16, tag="xn", name="xn2_bf")
            nc.vector.tensor_scalar(
                out=xn_bf,
                in0=x1[rt],
                scalar1=mv3[:, 0:1],
                scalar2=rstd3,
                op0=OP.subtract,
                op1=OP.mult,
            )
            nc.vector.tensor_add(out=x1[rt], in0=x1[rt], in1=b2_bc)
            for ft in range(FT):
                pt = ps_p6.tile([128, 128], bf16, tag="tp2", bufs=2, name="pt2")
                nc.tensor.transpose(pt, xn_bf[:, ft * 128 : (ft + 1) * 128], ident)
                nc.scalar.activation(
                    out=h2T[ft][:, rt * 128 : (rt + 1) * 128],
                    in_=pt,
                    func=AF.Identity,
                    bias=BfC[:, ft : ft + 1],
                    scale=WfC[:, ft : ft + 1],
                )

        cm_ps_p6.__exit__(None, None, None)

        cm_ps_mlp = tc.tile_pool(name="ps_mlp", bufs=1, space="PSUM")
        ps_mlp = cm_ps_mlp.__enter__()

        # ---------------- phase 7: mlp1 + gelu ----------------
        g1T = [mlps.tile([128, R], bf16, name=f"g1T{mt}") for mt in range(MFT)]
        for mt in range(MFT):
            w1blk = wstr.tile([128, FT * 128], bf16, tag="w1b", bufs=5, name="w1blk")
            nc.sync.dma_start(out=w1blk, in_=w1_d[mt])
            pg = ps_mlp.tile([128, 512], f32, tag="pg", bufs=3, name="pg")
            for kt in range(FT):
                nc.tensor.matmul(
                    pg,
                    w1blk[:, kt * 128 : (kt + 1) * 128],
                    h2T[kt],
                    start=(kt == 0),
                    stop=(kt == FT - 1),
                )
            nc.scalar.activation(
                out=g1T[mt],
                in_=pg,
                func=AF.Gelu,
                bias=b1_sb[:, mt : mt + 1],
                scale=1.0,
            )

        # ---------------- phase 8: mlp2 + residual -> y ----------------
        # mlp2's kt-th accumulation step only needs g1T[kt], so the fh=0
        # column sweep pipelines with mlp1 on the PE.
        for fh in range(2):
            pf = {}
            for rt in range(RB):
                pf[rt] = ps_mlp.tile(
                    [128, 512], f32, tag=f"pf{rt}", bufs=1, name=f"pf{rt}"
                )
            for kt in range(MFT):
                w2c = wstr.tile([128, 512], bf16, tag="w2c", bufs=6, name="w2c")
                eng = nc.gpsimd if kt % 2 == 0 else nc.scalar
                eng.dma_start(
                    out=w2c, in_=w2_d[kt * 128 : (kt + 1) * 128, fh * 512 : (fh + 1) * 512]
                )
                for rt in range(RB):
                    nc.tensor.matmul(
                        pf[rt],
                        g1T[kt][:, rt * 128 : (rt + 1) * 128],
                        w2c,
                        start=(kt == 0),
                        stop=(kt == MFT - 1),
                    )
            for rt in range(RB):
                sl = slice(fh * 512, (fh + 1) * 512)
                yh = work.tile([128, 512], f32, tag="yh", bufs=2, name="yh")
                nc.vector.tensor_add(out=yh, in0=pf[rt], in1=x1[rt][:, sl])
                nc.sync.dma_start(out=y_d[rt * 128 : (rt + 1) * 128, sl], in_=yh)

        cm_ps_mlp.__exit__(None, None, None)
        cm_mlps.__exit__(None, None, None)

    nc.compile()
    return nc


def _prep_in_maps(inputs):
    f32 = np.float32
    wmod = np.concatenate(
        [inputs["amod_gw"], inputs["amod_bw"], inputs["fmod_gw"], inputs["fmod_bw"]],
        axis=1,
    ).astype(BF16)
    modb = np.concatenate(
        [inputs["amod_gb"], inputs["amod_bb"], inputs["fmod_gb"], inputs["fmod_bb"]]
    ).astype(f32)
    lnvec = np.stack(
        [
            inputs["amod_nw"],
            inputs["amod_nb"],
            inputs["attn_nw"],
            inputs["attn_nb"],
            inputs["fmod_nw"],
            inputs["fmod_nb"],
        ]
    ).astype(f32)
    wq_t = np.ascontiguousarray(
        np.asarray(inputs["wq"]).astype(BF16).reshape(FT, 128, MT, 128)
        .transpose(2, 1, 0, 3).reshape(MT, 128, FT * 128)
    )
    w1_t = np.ascontiguousarray(
        np.asarray(inputs["w1"]).astype(BF16).reshape(FT, 128, MFT, 128)
        .transpose(2, 1, 0, 3).reshape(MFT, 128, FT * 128)
    )
    sel2 = np.zeros((128, H * 64), BF16)
    for hp2 in range(H // 2):
        sel2[2 * hp2, hp2 * 128 : hp2 * 128 + 64] = 1
        sel2[2 * hp2 + 1, hp2 * 128 + 64 : hp2 * 128 + 128] = 1
    shared = dict(
        sel2=sel2,
        wmod=wmod,
        modb=modb,
        lnvec=lnvec,
        wq=wq_t,
        wkv=np.ascontiguousarray(
            np.asarray(inputs["wkv"]).astype(BF16).reshape(FT, 128, 2 * D)
            .transpose(1, 0, 2)
        ),
        wo=np.asarray(inputs["wo"]).astype(BF16),
        wo_bias=np.asarray(inputs["wo_b"]).astype(f32).reshape(1, F),
        w1=w1_t,
        b1=np.asarray(inputs["b1"]).astype(f32),
        w2=np.asarray(inputs["w2"]).astype(BF16),
        b2=np.asarray(inputs["b2"]).astype(f32).reshape(1, F),
    )
    x = np.asarray(inputs["x"]).astype(f32)
    cond = np.asarray(inputs["cond"]).astype(BF16)
    in_maps = []
    for c in range(NCORES):
        b, r0 = c // 4, (c % 4) * R
        m = dict(shared)
        m["x"] = np.ascontiguousarray(x[b, r0 : r0 + R, :])
        m["cond"] = np.ascontiguousarray(cond[b])
        in_maps.append(m)
    return in_maps


def _run(inputs, trace=False):
    from concourse.bass_utils import run_bass_kernel_spmd

    if "nc" not in _CACHE:
        _CACHE["nc"] = _build_nc()
    nc = _CACHE["nc"]
    in_maps = _prep_in_maps(inputs)
    res = run_bass_kernel_spmd(
        nc, in_maps, core_ids=list(range(NCORES)), trace=trace
    )
    y = np.empty((B, T, F), np.float32)
    for c in range(NCORES):
        b, r0 = c // 4, (c % 4) * R
        y[b, r0 : r0 + R, :] = res.results[c]["y"]
    return y, res


def kernel(**inputs) -> np.ndarray:
    y, _ = _run(inputs, trace=False)
    return y


if __name__ == "__main__":
    _build_nc()
    print("build OK")


# revision 13
# speedup vs baseline: 1.8023x; 1.0463x over previous
"""DiT block kernel for 8x Trainium2 NeuronCores (Bass/Tile).

Sharding: row-parallel over the flattened (B,T)=4096 rows; 512 rows/core.
Cores 0-3 handle batch 0, cores 4-7 batch 1. MQA K/V is computed per-shard
and AllGather'd within each 4-core batch group. Weights are replicated and
cast to bf16; LN/residual math stays fp32.

v2 structure notes (driven by trace analysis of v1):
  - ALL attention matmuls run in the PE's (128,128) tile mode: QK^T uses
    zero-padded kT tiles (kT_lo has k on partitions 0-63 and zeros above,
    kT_hi the reverse) so the contraction is always 128-wide.  v1 alternated
    (64,128) MM1s with (128,128) PV matmuls instruction-by-instruction and
    every matmul paid a PE mode-switch drain (~2.5x slowdown).
  - exp() is applied to [128,1024] PSUM chunks (2 key tiles at once) to
    amortize the ~370ns ACT SBUF/PSUM access latency per instruction.
  - softmax denominators are gathered (via tiny PSUM->SBUF DMAs) into one
    [16,512] tile and inverted with ONE DVE reciprocal (v1: 16 calls at
    2.4us each), then broadcast per head-pair with a single (128,128)-mode
    select-matmul.
  - attention outputs land directly in head-pair-stacked tiles [128,R]
    (even head on partitions 0-63, odd on 64-127, via a dual-layout v_ext)
    so the out-projection contracts 128 partitions per matmul (v1: 64).
  - the adaLN-2 / attn-LN scale+bias are folded into the PSUM->SBUF copies
    of the transposes (per-partition scale/bias APs on ACT).
  - mod vectors, LN chains, DMA queue placement all restructured so the
    K/V AllGather is issued ~30us into the kernel instead of ~125us.
"""

import sys

sys.path.insert(0, "/opt/trn_rl_repo")

import numpy as np
import ml_dtypes

BF16 = ml_dtypes.bfloat16

B, T, F, H, D, M, C = 2, 2048, 1024, 16, 64, 4, 512
NCORES = 8
R = (B * T) // NCORES  # 512 rows per core
RB = R // 128  # 4 row blocks
FT = F // 128  # 8 feature tiles
MT = (H * D) // 128  # 8 head-pair tiles
MFT = (M * F) // 128  # 32 mlp hidden tiles
KT = T // 128  # 16 key tiles
EPS = 1e-5

_CACHE = {}


def _build_nc():
    import concourse.bass as bass
    import concourse.tile as tile
    from concourse import bacc, mybir
    from concourse.masks import make_identity
    from contextlib import ExitStack

    f32 = mybir.dt.float32
    f16 = mybir.dt.float16
    bf16 = mybir.dt.bfloat16
    AF = mybir.ActivationFunctionType
    OP = mybir.AluOpType

    nc = bacc.Bacc(
        "TRN2",
        target_bir_lowering=False,
        debug=False,
        enable_asserts=False,
        num_devices=NCORES,
    )

    def dram(name, shape, dt, **kw):
        return nc.dram_tensor(name, shape, dt, **kw).ap()

    x_d = dram("x", [R, F], f32, kind="ExternalInput")
    cond_d = dram("cond", [C], bf16, kind="ExternalInput")
    wmod_d = dram("wmod", [C, 4 * F], bf16, kind="ExternalInput")
    modb_d = dram("modb", [4 * F], f32, kind="ExternalInput")
    lnv_d = dram("lnvec", [6, F], f32, kind="ExternalInput")
    wq_d = dram("wq", [MT, 128, FT * 128], bf16, kind="ExternalInput")
    wkv_d = dram("wkv", [128, FT, 2 * D], bf16, kind="ExternalInput")
    wo_d = dram("wo", [H * D, F], bf16, kind="ExternalInput")
    wob_d = dram("wo_bias", [1, F], f32, kind="ExternalInput")
    w1_d = dram("w1", [MFT, 128, FT * 128], bf16, kind="ExternalInput")
    b1_d = dram("b1", [M * F], f32, kind="ExternalInput")
    w2_d = dram("w2", [M * F, F], bf16, kind="ExternalInput")
    b2_d = dram("b2", [1, F], f32, kind="ExternalInput")
    sel2_d = dram("sel2", [128, H * 64], bf16, kind="ExternalInput")
    y_d = dram("y", [R, F], f32, kind="ExternalOutput")

    groups = [[0, 1, 2, 3], [4, 5, 6, 7]]

    def bcast_row(ap_row):
        # [1, n] DRAM AP -> partition-broadcast [128, n]
        return bass.AP(
            tensor=ap_row.tensor,
            offset=ap_row.offset,
            ap=[[0, 128]] + list(ap_row.ap[-1:]),
        )

    def row_cols(ap2d, r):
        # row r of a [*, F] DRAM AP, viewed as [128, FT] columns:
        # out[p, t] = row[t*128 + p]
        row = bass.AP(
            tensor=ap2d.tensor,
            offset=ap2d.offset + r * F,
            ap=[[1, F]],
        )
        return row.rearrange("(t p) -> p t", p=128)

    with tile.TileContext(nc) as tc, ExitStack() as ctx:
        consts = ctx.enter_context(tc.tile_pool(name="consts", bufs=1))
        work = ctx.enter_context(tc.tile_pool(name="work", bufs=2))
        persist = ctx.enter_context(tc.tile_pool(name="persist", bufs=1))
        wstr = ctx.enter_context(tc.tile_pool(name="wstr", bufs=3))
        dramp = ctx.enter_context(tc.tile_pool(name="dramp", bufs=1, space="DRAM"))

        # ---------------- constants ----------------
        ident = consts.tile([128, 128], bf16, name="ident")
        make_identity(nc, ident)
        epst = consts.tile([128, 1], f32, name="epst")
        nc.vector.memset(epst, EPS)

        cond_sb = consts.tile([128, 4], bf16, name="cond_sb")
        nc.sync.dma_start(out=cond_sb, in_=cond_d.rearrange("(a p) -> p a", p=128))
        b1_sb = consts.tile([128, MFT], f32, name="b1_sb")
        nc.gpsimd.dma_start(out=b1_sb, in_=b1_d.rearrange("(mt p) -> p mt", p=128))
        wkv_sb = consts.tile([128, FT, 2 * D], bf16, name="wkv_sb")
        nc.gpsimd.dma_start(out=wkv_sb, in_=wkv_d)
        # attn-LN weight/bias in column layout (per-partition scale/bias of
        # the transpose copies)
        anwT = consts.tile([128, FT], f32, name="anwT")
        nc.gpsimd.dma_start(out=anwT, in_=row_cols(lnv_d, 2))
        anbT = consts.tile([128, FT], f32, name="anbT")
        nc.gpsimd.dma_start(out=anbT, in_=row_cols(lnv_d, 3))
        wob_bc = consts.tile([128, F], f32, name="wob_bc")
        nc.gpsimd.dma_start(out=wob_bc, in_=bcast_row(wob_d[0:1, :]))
        b2_bc = consts.tile([128, F], f32, name="b2_bc")
        nc.gpsimd.dma_start(out=b2_bc, in_=bcast_row(b2_d[0:1, :]))

        # head-pair select matrix for the softmax-denominator broadcast
        # (host-built: block hp2 selects den row 2*hp2 into output partitions
        # 0-63 and row 2*hp2+1 into partitions 64-127)
        sel2 = consts.tile([128, H * 64], bf16, name="sel2")
        nc.gpsimd.dma_start(out=sel2, in_=sel2_d)

        # phase-scoped SBUF pools, opened in LIFO-compatible nesting order
        # (innermost closes first): hera > attnp > midp > [psum scopes]
        cm_hera = tc.tile_pool(name="hera", bufs=1)  # hT
        hera = cm_hera.__enter__()
        cm_attn = tc.tile_pool(name="attnp", bufs=1)  # attention-era tiles
        attnp = cm_attn.__enter__()
        cm_mid = tc.tile_pool(name="midp", bufs=1)  # h_res/out2/wo2 era
        midp = cm_mid.__enter__()

        # PSUM pool for the startup phases (mod / transposes / kv / q / v_ext)
        cm_ps_start = tc.tile_pool(name="ps_start", bufs=1, space="PSUM")
        ps_start = cm_ps_start.__enter__()

        # ---------------- phase 0: modulation vectors ----------------
        # modv = cond @ [gA | bA | gF | bF] + modb, kept in a row-chunked
        # [8, 512] layout (chunk nb on partition nb) so the finalize math
        # runs on multiple DVE lanes and SBUF address-space cost stays tiny.
        # Wa/Ba are then round-tripped through DRAM into partition-broadcast
        # [128,F] tiles; Wf/Bf into column [128,FT] tiles.
        cm_modtmp = tc.tile_pool(name="modtmp", bufs=1)
        modtmp = cm_modtmp.__enter__()

        def row_chunked(ap2d, r, p):
            # row r of a [*, F-or-4F] DRAM AP as [p, n/p] (chunk c on lane c)
            n = ap2d.ap[-1][1]
            row = bass.AP(
                tensor=ap2d.tensor,
                offset=ap2d.offset + r * n,
                ap=[[1, n]],
            )
            return row.rearrange("(p j) -> p j", p=p)

        lnr8 = {}
        for r in (0, 1, 4, 5):  # amod_nw/nb, fmod_nw/nb as [2, 512]
            lnr8[r] = modtmp.tile([2, 512], f32, name=f"lnr8_{r}")
            nc.scalar.dma_start(out=lnr8[r], in_=row_chunked(lnv_d, r, 2))
        # four [2,512] vectors (gA, bA, gF, bF), each at base partition 0
        # (compute engines require base partition in {0,32,64,96})
        mvec = [modtmp.tile([2, 512], f32, name=f"mvec{v}") for v in range(4)]
        mbias = [modtmp.tile([2, 512], f32, name=f"mbias{v}") for v in range(4)]
        for v in range(4):
            nc.scalar.dma_start(
                out=mbias[v],
                in_=row_chunked(modb_d.rearrange("(a f) -> a f", a=4), v, 2),
            )
        tmp2 = modtmp.tile([2, 512], f32, name="tmp2")
        mod_dram = dramp.tile([4, F], f32, name="mod_dram")

        def mod_half(nbs, gi, bi, nw_r, nb_r):
            half_off = nbs[0] * 512
            wm_tiles = []
            for ch in range(4):
                wm = modtmp.tile(
                    [128, 2048], bf16, tag="wmbig", bufs=4, name=f"wm{ch}"
                )
                eng = nc.sync if ch % 2 == 0 else nc.gpsimd
                eng.dma_start(
                    out=wm,
                    in_=wmod_d[
                        ch * 128 : (ch + 1) * 128, half_off : half_off + 2048
                    ],
                )
                wm_tiles.append(wm)
            for nb in nbs:
                j = nb - nbs[0]
                pm = ps_start.tile([128, 512], f32, tag="sp", bufs=3, name="pm")
                for ch in range(4):
                    nc.tensor.matmul(
                        pm[0:1, :],
                        cond_sb[:, ch : ch + 1],
                        wm_tiles[ch][:, j * 512 : (j + 1) * 512],
                        start=(ch == 0),
                        stop=(ch == 3),
                    )
                pmrow = modtmp.tile([1, 512], f32, tag="pmrow", bufs=2, name="pmrow")
                nc.vector.tensor_copy(out=pmrow, in_=pm[0:1, :])
                # relocate chunk nb to lane nb%2 of vector nb//2
                nc.sync.dma_start(
                    out=mvec[nb // 2][nb % 2 : nb % 2 + 1, :], in_=pmrow
                )
            for v in (gi, bi):
                nc.vector.tensor_add(out=mvec[v], in0=mvec[v], in1=mbias[v])
            # finalize: g := nw*(1+g), b := nb*(1+g) + b
            nc.scalar.add(out=mvec[gi], in_=mvec[gi], add=1.0)
            nc.vector.tensor_mul(out=tmp2, in0=mvec[gi], in1=lnr8[nb_r])
            nc.vector.tensor_add(out=mvec[bi], in0=tmp2, in1=mvec[bi])
            nc.vector.tensor_mul(out=mvec[gi], in0=mvec[gi], in1=lnr8[nw_r])
            for v in (gi, bi):
                nc.sync.dma_start(out=row_chunked(mod_dram, v, 2), in_=mvec[v])

        # amod half first: Wa_bc/Ba_bc gate all of phase 1
        mod_half((0, 1, 2, 3), 0, 1, 0, 1)
        bc = {}
        for v, nm in ((0, "Wa_bc"), (1, "Ba_bc")):
            bt = consts.tile([128, F], f32, name=nm)
            nc.sync.dma_start(out=bt, in_=bcast_row(mod_dram[v : v + 1, :]))
            bc[nm] = bt
        # fmod half (consumed only at phase 6)
        mod_half((4, 5, 6, 7), 2, 3, 4, 5)
        WfC = consts.tile([128, FT], f32, name="WfC")
        nc.sync.dma_start(out=WfC, in_=row_cols(mod_dram, 2))
        BfC = consts.tile([128, FT], f32, name="BfC")
        nc.sync.dma_start(out=BfC, in_=row_cols(mod_dram, 3))

        cm_modtmp.__exit__(None, None, None)

        # ---------------- LN stats helper ----------------
        def ln_stats(src):
            stats = work.tile([128, 2, 6], f32, tag="stats", name="stats")
            for sg in range(2):
                nc.vector.bn_stats(
                    out=stats[:, sg, :], in_=src[:, sg * 512 : (sg + 1) * 512]
                )
            mv = work.tile([128, 2], f32, tag="mv", name="mv")
            nc.vector.bn_aggr(out=mv, in_=stats)
            rstd = work.tile([128, 1], f32, tag="rstd", name="rstd")
            nc.scalar.activation(
                out=rstd, in_=mv[:, 1:2], func=AF.Sqrt, bias=epst, scale=1.0
            )
            nc.vector.reciprocal(out=rstd, in_=rstd)
            return mv, rstd

        hT = [hera.tile([128, R], bf16, name=f"hT{ft}") for ft in range(FT)]
        h_res = [midp.tile([128, F], bf16, name=f"h{rb}") for rb in range(RB)]

        # ---------------- phase 1: adaLN-1 + attn-LN + transpose ----------------
        for rb in range(RB):
            x_rb = work.tile([128, F], f32, tag="x", name="x_rb")
            nc.scalar.dma_start(out=x_rb, in_=x_d[rb * 128 : (rb + 1) * 128, :])
            mv1, rstd1 = ln_stats(x_rb)
            nc.vector.scalar_tensor_tensor(
                out=x_rb,
                in0=x_rb,
                scalar=mv1[:, 0:1],
                in1=bc["Wa_bc"],
                op0=OP.subtract,
                op1=OP.mult,
            )
            nc.vector.scalar_tensor_tensor(
                out=h_res[rb],
                in0=x_rb,
                scalar=rstd1,
                in1=bc["Ba_bc"],
                op0=OP.mult,
                op1=OP.add,
            )
            mv2, rstd2 = ln_stats(h_res[rb])
            xn_bf = work.tile([128, F], bf16, tag="xn", name="xn_bf")
            nc.vector.tensor_scalar(
                out=xn_bf,
                in0=h_res[rb],
                scalar1=mv2[:, 0:1],
                scalar2=rstd2,
                op0=OP.subtract,
                op1=OP.mult,
            )
            for ft in range(FT):
                pt = ps_start.tile([128, 128], bf16, tag="tp", bufs=2, name="pt")
                nc.tensor.transpose(pt, xn_bf[:, ft * 128 : (ft + 1) * 128], ident)
                nc.scalar.activation(
                    out=hT[ft][:, rb * 128 : (rb + 1) * 128],
                    in_=pt,
                    func=AF.Identity,
                    bias=anbT[:, ft : ft + 1],
                    scale=anwT[:, ft : ft + 1],
                )

        # ---------------- phase 2: kv first (AllGather ASAP), then q ----------------
        pkv = ps_start.tile([128, 512], f32, tag="sp", bufs=3, name="pkv")
        for kt in range(FT):
            nc.tensor.matmul(
                pkv, wkv_sb[:, kt, :], hT[kt], start=(kt == 0), stop=(kt == FT - 1)
            )
        kvT_sb = work.tile([128, R], bf16, tag="kvT", bufs=1, name="kvT_sb")
        nc.scalar.activation(out=kvT_sb, in_=pkv, func=AF.Copy)

        kvT_bounce = dramp.tile([2 * D, R], bf16, name="kvT_bounce")
        kvT_all = dramp.tile([4 * 2 * D, R], bf16, name="kvT_all")
        nc.sync.dma_start(out=kvT_bounce, in_=kvT_sb)
        nc.gpsimd.collective_compute(
            "AllGather",
            OP.bypass,
            replica_groups=groups,
            ins=[kvT_bounce[:, :]],
            outs=[kvT_all[:, :]],
        )

        # q projection fills the AllGather wait
        qT = [attnp.tile([128, R], bf16, name=f"qT{mt}") for mt in range(MT)]
        for mt in range(MT):
            wqblk = wstr.tile([128, FT * 128], bf16, tag="wqb", bufs=3, name="wqblk")
            nc.scalar.dma_start(out=wqblk, in_=wq_d[mt])
            pq = ps_start.tile([128, 512], f32, tag="sp", bufs=3, name="pq")
            for kt in range(FT):
                nc.tensor.matmul(
                    pq,
                    wqblk[:, kt * 128 : (kt + 1) * 128],
                    hT[kt],
                    start=(kt == 0),
                    stop=(kt == FT - 1),
                )
            # fold the attention 1/sqrt(D)=0.125 scale into q
            nc.scalar.activation(out=qT[mt], in_=pq, func=AF.Copy, scale=0.125)

        # fold the out-proj bias into the residual while the AllGather is in
        # flight (x1 = px + (h_res + wo_b))
        for rb in range(RB):
            nc.vector.tensor_add(out=h_res[rb], in0=h_res[rb], in1=wob_bc)

        # preload exp's ACT table set during the AllGather window
        warm = work.tile([1, 1], f32, tag="warm", bufs=1, name="warm")
        nc.scalar.activation(out=warm, in_=epst[0:1, 0:1], func=AF.Exp)

        # prefetch wo (consumed in phase 5)
        wo2sb = [midp.tile([128, F], bf16, name=f"wo2_{hp}") for hp in range(MT)]
        for hp in range(MT):
            nc.gpsimd.dma_start(out=wo2sb[hp], in_=wo_d[hp * 128 : (hp + 1) * 128, :])

        # ---------------- phase 3: kT / v_ext assembly ----------------
        # kT_lo: k on partitions 0-63, zeros above  -> even heads (q on 0-63)
        # kT_hi: k on partitions 64-127, zeros below -> odd heads
        # Both give contraction-128 matmuls: no PE tile-mode switching.
        # v_ext dual layout [128, 192]:
        #   cols 0-63 = v dims, col 64 = ones, cols 65-127 = 0, cols 128-191 = v dims
        # even heads use lhsT [:,0:65]  -> po dims at partitions 0-63, den at 64
        # odd  heads use lhsT [:,64:192]-> po den at partition 0, dims at 64-127
        v_ext = [attnp.tile([128, 192], bf16, name=f"vext{kt}") for kt in range(KT)]
        for kt in range(KT):
            nc.vector.memset(v_ext[kt][:, 64:128], 0.0)
            nc.vector.memset(v_ext[kt][:, 64:65], 1.0)
        for r in range(4):
            vT_sb = work.tile([64, R], bf16, tag="vTs", bufs=1, name="vT_sb")
            nc.scalar.dma_start(
                out=vT_sb, in_=kvT_all[r * 128 + 64 : (r + 1) * 128, :]
            )
            for cc in range(4):
                kt = r * 4 + cc
                ptv = ps_start.tile([128, 128], bf16, tag="tp", bufs=2, name="ptv")
                nc.tensor.matmul(
                    ptv[:, 0:64],
                    vT_sb[:, cc * 128 : (cc + 1) * 128],
                    ident[0:64, 0:64],
                    is_transpose=True,
                )
                nc.vector.tensor_copy(out=v_ext[kt][:, 0:64], in_=ptv[:, 0:64])
                nc.vector.tensor_copy(out=v_ext[kt][:, 128:192], in_=ptv[:, 0:64])

        kT_lo = attnp.tile([128, T], bf16, name="kT_lo")
        kT_hi = attnp.tile([128, T], bf16, name="kT_hi")
        nc.vector.memset(kT_lo[64:128, :], 0.0)
        nc.vector.memset(kT_hi[0:64, :], 0.0)
        for r in range(4):
            nc.sync.dma_start(
                out=kT_lo[0:64, r * R : (r + 1) * R],
                in_=kvT_all[r * 128 : r * 128 + 64, :],
            )
            nc.gpsimd.dma_start(
                out=kT_hi[64:128, r * R : (r + 1) * R],
                in_=kvT_all[r * 128 : r * 128 + 64, :],
            )
        den16 = attnp.tile([16, R], f32, name="den16")
        rcpz = attnp.tile([128, R], bf16, name="rcpz")
        nc.vector.memset(rcpz, 0.0)

        cm_ps_start.__exit__(None, None, None)

        # attention-era PSUM: 3 x [128,1024] (6 banks) + 2 x po (2 banks)
        cm_ps_attn = tc.tile_pool(name="ps_attn", bufs=1, space="PSUM")
        ps_attn = cm_ps_attn.__enter__()

        # out2[hp2]: attention output stacked by head pair (head 2*hp2 on
        # partitions 0-63, head 2*hp2+1 on 64-127), raw (pre-normalization).
        out2 = [midp.tile([128, R], bf16, name=f"out2_{hp}") for hp in range(H // 2)]

        # ---------------- phase 4: attention ----------------
        for h in range(H):
            mt, even = h // 2, (h % 2) == 0
            kTs = kT_lo if even else kT_hi
            po = ps_attn.tile([128, 512], f32, tag="po", bufs=2, name="po")
            for c in range(KT // 2):
                psc = ps_attn.tile([128, 1024], f32, tag="ps2", bufs=3, name="psc")
                for half in range(2):
                    kt = 2 * c + half
                    nc.tensor.matmul(
                        psc[:, half * 512 : (half + 1) * 512],
                        kTs[:, kt * 128 : (kt + 1) * 128],
                        qT[mt],
                        start=True,
                        stop=True,
                    )
                pr = attnp.tile([128, 1024], bf16, tag="pr", bufs=4, name="pr")
                nc.scalar.activation(out=pr, in_=psc, func=AF.Exp)
                for half in range(2):
                    kt = 2 * c + half
                    lhs = v_ext[kt][:, 0:65] if even else v_ext[kt][:, 64:192]
                    outsl = po[0:65, :] if even else po[0:128, :]
                    nc.tensor.matmul(
                        outsl,
                        lhs,
                        pr[:, half * 512 : (half + 1) * 512],
                        start=(c == 0 and half == 0),
                        stop=(c == KT // 2 - 1 and half == 1),
                    )
            # stage the denominator row to SBUF (same-partition DVE copy),
            # then cross-partition SBUF->SBUF DMA into the den16 gather tile
            stg = work.tile([128, 512], f32, tag="dstg", name="dstg")
            if even:
                nc.vector.tensor_copy(out=stg[64:65, :], in_=po[64:65, :])
                nc.gpsimd.dma_start(out=den16[h : h + 1, :], in_=stg[64:65, :])
                nc.vector.tensor_copy(out=out2[mt][0:64, :], in_=po[0:64, :])
            else:
                nc.vector.tensor_copy(out=stg[0:1, :], in_=po[0:1, :])
                nc.gpsimd.dma_start(out=den16[h : h + 1, :], in_=stg[0:1, :])
                nc.vector.tensor_copy(out=out2[mt][64:128, :], in_=po[64:128, :])

        cm_ps_attn.__exit__(None, None, None)

        cm_ps_p5 = tc.tile_pool(name="ps_p5", bufs=1, space="PSUM")
        ps_p5 = cm_ps_p5.__enter__()

        # ---------------- phase 5: normalize + out proj + residual -> x1 ----------------
        with nc.allow_low_precision(reason="bf16 softmax denom broadcast"):
            nc.vector.reciprocal(out=rcpz[0:16, :], in_=den16)
        for hp2 in range(H // 2):
            bcrt = ps_p5.tile([128, 512], f32, tag="bcr", bufs=2, name="bcrt")
            nc.tensor.matmul(
                bcrt, sel2[:, hp2 * 128 : (hp2 + 1) * 128], rcpz, start=True, stop=True
            )
            nc.vector.tensor_mul(out=out2[hp2], in0=out2[hp2], in1=bcrt)

        x1 = [persist.tile([128, F], f32, name=f"x1_{rt}") for rt in range(RB)]
        for rt in range(RB):
            for fh in range(2):
                px = ps_p5.tile([128, 512], f32, tag="px", bufs=3, name="px")
                for hp2 in range(H // 2):
                    nc.tensor.matmul(
                        px,
                        out2[hp2][:, rt * 128 : (rt + 1) * 128],
                        wo2sb[hp2][:, fh * 512 : (fh + 1) * 512],
                        start=(hp2 == 0),
                        stop=(hp2 == H // 2 - 1),
                    )
                sl = slice(fh * 512, (fh + 1) * 512)
                nc.vector.tensor_add(out=x1[rt][:, sl], in0=px, in1=h_res[rt][:, sl])

        cm_ps_p5.__exit__(None, None, None)
        cm_mid.__exit__(None, None, None)
        cm_attn.__exit__(None, None, None)
        cm_hera.__exit__(None, None, None)

        cm_ps_p6 = tc.tile_pool(name="ps_p6", bufs=1, space="PSUM")
        ps_p6 = cm_ps_p6.__enter__()

        cm_mlps = tc.tile_pool(name="mlps", bufs=1)
        mlps = cm_mlps.__enter__()

        # ---------------- phase 6: adaLN-2 + transpose ----------------
        # LN core on DVE; Wf/Bf applied as per-partition scale/bias in the
        # PSUM->SBUF transpose copies. b2 is folded into x1 right after its
        # LN stats are consumed (y = mlp2 + (x1 + b2)).
        h2T = [mlps.tile([128, R], bf16, name=f"h2T{ft}") for ft in range(FT)]
        for rt in range(RB):
            mv3, rstd3 = ln_stats(x1[rt])
            xn_bf = work.tile([128, F], bf16, tag="xn", name="xn2_bf")
            nc.vector.tensor_scalar(
                out=xn_bf,
                in0=x1[rt],
                scalar1=mv3[:, 0:1],
                scalar2=rstd3,
                op0=OP.subtract,
                op1=OP.mult,
            )
            nc.vector.tensor_add(out=x1[rt], in0=x1[rt], in1=b2_bc)
            for ft in range(FT):
                pt = ps_p6.tile([128, 128], bf16, tag="tp2", bufs=2, name="pt2")
                nc.tensor.transpose(pt, xn_bf[:, ft * 128 : (ft + 1) * 128], ident)
                nc.scalar.activation(
                    out=h2T[ft][:, rt * 128 : (rt + 1) * 128],
                    in_=pt,
                    func=AF.Identity,
                    bias=BfC[:, ft : ft + 1],
                    scale=WfC[:, ft : ft + 1],
                )

        cm_ps_p6.__exit__(None, None, None)

        cm_ps_mlp = tc.tile_pool(name="ps_mlp", bufs=1, space="PSUM")
        ps_mlp = cm_ps_mlp.__enter__()

        # ---------------- phase 7: mlp1 + gelu ----------------
        g1T = [mlps.tile([128, R], bf16, name=f"g1T{mt}") for mt in range(MFT)]
        for mt in range(MFT):
            w1blk = wstr.tile([128, FT * 128], bf16, tag="w1b", bufs=3, name="w1blk")
            nc.sync.dma_start(out=w1blk, in_=w1_d[mt])
            pg = ps_mlp.tile([128, 512], f32, tag="pg", bufs=3, name="pg")
            for kt in range(FT):
                nc.tensor.matmul(
                    pg,
                    w1blk[:, kt * 128 : (kt + 1) * 128],
                    h2T[kt],
                    start=(kt == 0),
                    stop=(kt == FT - 1),
                )
            nc.scalar.activation(
                out=g1T[mt],
                in_=pg,
                func=AF.Gelu,
                bias=b1_sb[:, mt : mt + 1],
                scale=1.0,
            )

        # ---------------- phase 8: mlp2 + residual -> y ----------------
        # mlp2's kt-th accumulation step only needs g1T[kt], so the fh=0
        # column sweep pipelines with mlp1 on the PE.
        for fh in range(2):
            pf = {}
            for rt in range(RB):
                pf[rt] = ps_mlp.tile(
                    [128, 512], f32, tag=f"pf{rt}", bufs=1, name=f"pf{rt}"
                )
            for kt in range(MFT):
                w2c = wstr.tile([128, 512], bf16, tag="w2c", bufs=6, name="w2c")
                eng = nc.gpsimd if kt % 2 == 0 else nc.scalar
                eng.dma_start(
                    out=w2c, in_=w2_d[kt * 128 : (kt + 1) * 128, fh * 512 : (fh + 1) * 512]
                )
                for rt in range(RB):
                    nc.tensor.matmul(
                        pf[rt],
                        g1T[kt][:, rt * 128 : (rt + 1) * 128],
                        w2c,
                        start=(kt == 0),
                        stop=(kt == MFT - 1),
                    )
            for rt in range(RB):
                sl = slice(fh * 512, (fh + 1) * 512)
                yh = work.tile([128, 512], f32, tag="yh", bufs=2, name="yh")
                nc.vector.tensor_add(out=yh, in0=pf[rt], in1=x1[rt][:, sl])
                nc.sync.dma_start(out=y_d[rt * 128 : (rt + 1) * 128, sl], in_=yh)

        cm_ps_mlp.__exit__(None, None, None)
        cm_mlps.__exit__(None, None, None)

    nc.compile()
    return nc


def _prep_in_maps(inputs):
    f32 = np.float32
    wmod = np.concatenate(
        [inputs["amod_gw"], inputs["amod_bw"], inputs["fmod_gw"], inputs["fmod_bw"]],
        axis=1,
    ).astype(BF16)
    modb = np.concatenate(
        [inputs["amod_gb"], inputs["amod_bb"], inputs["fmod_gb"], inputs["fmod_bb"]]
    ).astype(f32)
    lnvec = np.stack(
        [
            inputs["amod_nw"],
            inputs["amod_nb"],
            inputs["attn_nw"],
            inputs["attn_nb"],
            inputs["fmod_nw"],
            inputs["fmod_nb"],
        ]
    ).astype(f32)
    wq_t = np.ascontiguousarray(
        np.asarray(inputs["wq"]).astype(BF16).reshape(FT, 128, MT, 128)
        .transpose(2, 1, 0, 3).reshape(MT, 128, FT * 128)
    )
    w1_t = np.ascontiguousarray(
        np.asarray(inputs["w1"]).astype(BF16).reshape(FT, 128, MFT, 128)
        .transpose(2, 1, 0, 3).reshape(MFT, 128, FT * 128)
    )
    sel2 = np.zeros((128, H * 64), BF16)
    for hp2 in range(H // 2):
        sel2[2 * hp2, hp2 * 128 : hp2 * 128 + 64] = 1
        sel2[2 * hp2 + 1, hp2 * 128 + 64 : hp2 * 128 + 128] = 1
    shared = dict(
        sel2=sel2,
        wmod=wmod,
        modb=modb,
        lnvec=lnvec,
        wq=wq_t,
        wkv=np.ascontiguousarray(
            np.asarray(inputs["wkv"]).astype(BF16).reshape(FT, 128, 2 * D)
            .transpose(1, 0, 2)
        ),
        wo=np.asarray(inputs["wo"]).astype(BF16),
        wo_bias=np.asarray(inputs["wo_b"]).astype(f32).reshape(1, F),
        w1=w1_t,
        b1=np.asarray(inputs["b1"]).astype(f32),
        w2=np.asarray(inputs["w2"]).astype(BF16),
        b2=np.asarray(inputs["b2"]).astype(f32).reshape(1, F),
    )
    x = np.asarray(inputs["x"]).astype(f32)
    cond = np.asarray(inputs["cond"]).astype(BF16)
    in_maps = []
    for c in range(NCORES):
        b, r0 = c // 4, (c % 4) * R
        m = dict(shared)
        m["x"] = np.ascontiguousarray(x[b, r0 : r0 + R, :])
        m["cond"] = np.ascontiguousarray(cond[b])
        in_maps.append(m)
    return in_maps


def _run(inputs, trace=False):
    from concourse.bass_utils import run_bass_kernel_spmd

    if "nc" not in _CACHE:
        _CACHE["nc"] = _build_nc()
    nc = _CACHE["nc"]
    in_maps = _prep_in_maps(inputs)
    res = run_bass_kernel_spmd(
        nc, in_maps, core_ids=list(range(NCORES)), trace=trace
    )
    y = np.empty((B, T, F), np.float32)
    for c in range(NCORES):
        b, r0 = c // 4, (c % 4) * R
        y[b, r0 : r0 + R, :] = res.results[c]["y"]
    return y, res


def kernel(**inputs) -> np.ndarray:
    y, _ = _run(inputs, trace=False)
    return y


if __name__ == "__main__":
    _build_nc()
    print("build OK")


# revision 15
# speedup vs baseline: 1.8403x; 1.0211x over previous
"""DiT block kernel for 8x Trainium2 NeuronCores (Bass/Tile).

Sharding: row-parallel over the flattened (B,T)=4096 rows; 512 rows/core.
Cores 0-3 handle batch 0, cores 4-7 batch 1. MQA K/V is computed per-shard
and AllGather'd within each 4-core batch group. Weights are replicated and
cast to bf16; LN/residual math stays fp32.

v2 structure notes (driven by trace analysis of v1):
  - ALL attention matmuls run in the PE's (128,128) tile mode: QK^T uses
    zero-padded kT tiles (kT_lo has k on partitions 0-63 and zeros above,
    kT_hi the reverse) so the contraction is always 128-wide.  v1 alternated
    (64,128) MM1s with (128,128) PV matmuls instruction-by-instruction and
    every matmul paid a PE mode-switch drain (~2.5x slowdown).
  - exp() is applied to [128,1024] PSUM chunks (2 key tiles at once) to
    amortize the ~370ns ACT SBUF/PSUM access latency per instruction.
  - softmax denominators are gathered (via tiny PSUM->SBUF DMAs) into one
    [16,512] tile and inverted with ONE DVE reciprocal (v1: 16 calls at
    2.4us each), then broadcast per head-pair with a single (128,128)-mode
    select-matmul.
  - attention outputs land directly in head-pair-stacked tiles [128,R]
    (even head on partitions 0-63, odd on 64-127, via a dual-layout v_ext)
    so the out-projection contracts 128 partitions per matmul (v1: 64).
  - the adaLN-2 / attn-LN scale+bias are folded into the PSUM->SBUF copies
    of the transposes (per-partition scale/bias APs on ACT).
  - mod vectors, LN chains, DMA queue placement all restructured so the
    K/V AllGather is issued ~30us into the kernel instead of ~125us.
"""

import sys

sys.path.insert(0, "/opt/trn_rl_repo")

import numpy as np
import ml_dtypes

BF16 = ml_dtypes.bfloat16

B, T, F, H, D, M, C = 2, 2048, 1024, 16, 64, 4, 512
NCORES = 8
R = (B * T) // NCORES  # 512 rows per core
RB = R // 128  # 4 row blocks
FT = F // 128  # 8 feature tiles
MT = (H * D) // 128  # 8 head-pair tiles
MFT = (M * F) // 128  # 32 mlp hidden tiles
KT = T // 128  # 16 key tiles
EPS = 1e-5

_CACHE = {}


def _build_nc():
    import concourse.bass as bass
    import concourse.tile as tile
    from concourse import bacc, mybir
    from concourse.masks import make_identity
    from contextlib import ExitStack

    f32 = mybir.dt.float32
    f16 = mybir.dt.float16
    bf16 = mybir.dt.bfloat16
    AF = mybir.ActivationFunctionType
    OP = mybir.AluOpType

    nc = bacc.Bacc(
        "TRN2",
        target_bir_lowering=False,
        debug=False,
        enable_asserts=False,
        num_devices=NCORES,
    )

    def dram(name, shape, dt, **kw):
        return nc.dram_tensor(name, shape, dt, **kw).ap()

    x_d = dram("x", [R, F], f32, kind="ExternalInput")
    cond_d = dram("cond", [C], bf16, kind="ExternalInput")
    wmod_d = dram("wmod", [C, 4 * F], bf16, kind="ExternalInput")
    # pack1: anwT(0:8) | anbT(8:16) | b1 columns(16:48), all [128, n] f32
    pack1_d = dram("pack1", [128, 48], f32, kind="ExternalInput")
    # pack2: lnr chunks (amod_nw/nb, fmod_nw/nb) then modb chunks, [2,512] each
    pack2_d = dram("pack2", [2, 8, 512], f32, kind="ExternalInput")
    wq_d = dram("wq", [MT, 128, FT * 128], bf16, kind="ExternalInput")
    wkv_d = dram("wkv", [128, FT, 2 * D], bf16, kind="ExternalInput")
    wo_d = dram("wo", [H * D, F], bf16, kind="ExternalInput")
    wob_d = dram("wo_bias", [1, F], f32, kind="ExternalInput")
    w1_d = dram("w1", [MFT, 128, FT * 128], bf16, kind="ExternalInput")
    w2_d = dram("w2", [M * F, F], bf16, kind="ExternalInput")
    b2_d = dram("b2", [1, F], f32, kind="ExternalInput")
    sel2_d = dram("sel2", [128, H * 64], bf16, kind="ExternalInput")
    y_d = dram("y", [R, F], f32, kind="ExternalOutput")

    groups = [[0, 1, 2, 3], [4, 5, 6, 7]]

    def bcast_row(ap_row):
        # [1, n] DRAM AP -> partition-broadcast [128, n]
        return bass.AP(
            tensor=ap_row.tensor,
            offset=ap_row.offset,
            ap=[[0, 128]] + list(ap_row.ap[-1:]),
        )

    def row_cols(ap2d, r):
        # row r of a [*, F] DRAM AP, viewed as [128, FT] columns:
        # out[p, t] = row[t*128 + p]
        row = bass.AP(
            tensor=ap2d.tensor,
            offset=ap2d.offset + r * F,
            ap=[[1, F]],
        )
        return row.rearrange("(t p) -> p t", p=128)

    with tile.TileContext(nc) as tc, ExitStack() as ctx:
        consts = ctx.enter_context(tc.tile_pool(name="consts", bufs=1))
        work = ctx.enter_context(tc.tile_pool(name="work", bufs=2))
        persist = ctx.enter_context(tc.tile_pool(name="persist", bufs=1))
        wstr = ctx.enter_context(tc.tile_pool(name="wstr", bufs=3))
        dramp = ctx.enter_context(tc.tile_pool(name="dramp", bufs=1, space="DRAM"))

        # ---------------- constants ----------------
        ident = consts.tile([128, 128], bf16, name="ident")
        make_identity(nc, ident)
        epst = consts.tile([128, 1], f32, name="epst")
        nc.vector.memset(epst, EPS)

        cond_sb = consts.tile([128, 4], bf16, name="cond_sb")
        nc.sync.dma_start(out=cond_sb, in_=cond_d.rearrange("(a p) -> p a", p=128))
        pack2 = consts.tile([2, 8, 512], f32, name="pack2")
        nc.scalar.dma_start(out=pack2, in_=pack2_d)
        lnr8 = {0: pack2[:, 0, :], 1: pack2[:, 1, :], 4: pack2[:, 2, :], 5: pack2[:, 3, :]}
        mbias = [pack2[:, 4 + v, :] for v in range(4)]

        # phase-scoped SBUF pools, opened in LIFO-compatible nesting order
        # (innermost closes first): hera > attnp > midp > [psum scopes]
        cm_hera = tc.tile_pool(name="hera", bufs=1)  # hT
        hera = cm_hera.__enter__()
        cm_attn = tc.tile_pool(name="attnp", bufs=1)  # attention-era tiles
        attnp = cm_attn.__enter__()
        cm_mid = tc.tile_pool(name="midp", bufs=1)  # h_res/out2/wo2 era
        midp = cm_mid.__enter__()

        # PSUM pool for the startup phases (mod / transposes / kv / q / v_ext)
        cm_ps_start = tc.tile_pool(name="ps_start", bufs=1, space="PSUM")
        ps_start = cm_ps_start.__enter__()

        # ---------------- phase 0: modulation vectors ----------------
        # modv = cond @ [gA | bA | gF | bF] + modb, kept in a row-chunked
        # [8, 512] layout (chunk nb on partition nb) so the finalize math
        # runs on multiple DVE lanes and SBUF address-space cost stays tiny.
        # Wa/Ba are then round-tripped through DRAM into partition-broadcast
        # [128,F] tiles; Wf/Bf into column [128,FT] tiles.
        cm_modtmp = tc.tile_pool(name="modtmp", bufs=1)
        modtmp = cm_modtmp.__enter__()

        def row_chunked(ap2d, r, p):
            # row r of a [*, F-or-4F] DRAM AP as [p, n/p] (chunk c on lane c)
            n = ap2d.ap[-1][1]
            row = bass.AP(
                tensor=ap2d.tensor,
                offset=ap2d.offset + r * n,
                ap=[[1, n]],
            )
            return row.rearrange("(p j) -> p j", p=p)

        # four [2,512] result vectors (gA, bA, gF, bF), base partition 0
        # (compute engines require base partition in {0,32,64,96})
        mvec = [modtmp.tile([2, 512], f32, name=f"mvec{v}") for v in range(4)]
        tmp2 = modtmp.tile([2, 512], f32, name="tmp2")
        mod_dram = dramp.tile([4, F], f32, name="mod_dram")

        def mod_half(nbs, gi, bi, nw_r, nb_r):
            half_off = nbs[0] * 512
            wm_tiles = []
            for ch in range(4):
                wm = modtmp.tile(
                    [128, 2048], bf16, tag="wmbig", bufs=4, name=f"wm{ch}"
                )
                eng = nc.sync if ch % 2 == 0 else nc.gpsimd
                eng.dma_start(
                    out=wm,
                    in_=wmod_d[
                        ch * 128 : (ch + 1) * 128, half_off : half_off + 2048
                    ],
                )
                wm_tiles.append(wm)
            for nb in nbs:
                j = nb - nbs[0]
                pm = ps_start.tile([128, 512], f32, tag="sp", bufs=3, name="pm")
                for ch in range(4):
                    nc.tensor.matmul(
                        pm[0:1, :],
                        cond_sb[:, ch : ch + 1],
                        wm_tiles[ch][:, j * 512 : (j + 1) * 512],
                        start=(ch == 0),
                        stop=(ch == 3),
                    )
                pmrow = modtmp.tile([1, 512], f32, tag="pmrow", bufs=2, name="pmrow")
                nc.vector.tensor_copy(out=pmrow, in_=pm[0:1, :])
                # relocate chunk nb to lane nb%2 of vector nb//2
                nc.sync.dma_start(
                    out=mvec[nb // 2][nb % 2 : nb % 2 + 1, :], in_=pmrow
                )
            for v in (gi, bi):
                nc.vector.tensor_add(out=mvec[v], in0=mvec[v], in1=mbias[v])
            # finalize: g := nw*(1+g), b := nb*(1+g) + b
            nc.scalar.add(out=mvec[gi], in_=mvec[gi], add=1.0)
            nc.vector.tensor_mul(out=tmp2, in0=mvec[gi], in1=lnr8[nb_r])
            nc.vector.tensor_add(out=mvec[bi], in0=tmp2, in1=mvec[bi])
            nc.vector.tensor_mul(out=mvec[gi], in0=mvec[gi], in1=lnr8[nw_r])
            for v in (gi, bi):
                nc.sync.dma_start(out=row_chunked(mod_dram, v, 2), in_=mvec[v])

        # amod half first: Wa_bc/Ba_bc gate all of phase 1
        mod_half((0, 1, 2, 3), 0, 1, 0, 1)
        bc = {}
        for v, nm in ((0, "Wa_bc"), (1, "Ba_bc")):
            bt = consts.tile([128, F], f32, name=nm)
            nc.sync.dma_start(out=bt, in_=bcast_row(mod_dram[v : v + 1, :]))
            bc[nm] = bt
        # fmod half (consumed only at phase 6)
        mod_half((4, 5, 6, 7), 2, 3, 4, 5)
        WfC = consts.tile([128, FT], f32, name="WfC")
        nc.sync.dma_start(out=WfC, in_=row_cols(mod_dram, 2))
        BfC = consts.tile([128, FT], f32, name="BfC")
        nc.sync.dma_start(out=BfC, in_=row_cols(mod_dram, 3))

        cm_modtmp.__exit__(None, None, None)

        # non-critical constants, loaded after the modulation path so their
        # DMA traffic doesn't delay the Wa/Ba round trip
        pack1 = consts.tile([128, 48], f32, name="pack1")
        nc.gpsimd.dma_start(out=pack1, in_=pack1_d)
        anwT = pack1[:, 0:FT]
        anbT = pack1[:, FT : 2 * FT]
        b1_sb = pack1[:, 2 * FT : 2 * FT + MFT]
        wkv_sb = consts.tile([128, FT, 2 * D], bf16, name="wkv_sb")
        nc.gpsimd.dma_start(out=wkv_sb, in_=wkv_d)
        sel2 = consts.tile([128, H * 64], bf16, name="sel2")
        nc.gpsimd.dma_start(out=sel2, in_=sel2_d)
        wob_bc = consts.tile([128, F], f32, name="wob_bc")
        nc.gpsimd.dma_start(out=wob_bc, in_=bcast_row(wob_d[0:1, :]))
        b2_bc = consts.tile([128, F], f32, name="b2_bc")
        nc.gpsimd.dma_start(out=b2_bc, in_=bcast_row(b2_d[0:1, :]))


        # ---------------- LN stats helper ----------------
        def ln_stats(src):
            stats = work.tile([128, 2, 6], f32, tag="stats", name="stats")
            for sg in range(2):
                nc.vector.bn_stats(
                    out=stats[:, sg, :], in_=src[:, sg * 512 : (sg + 1) * 512]
                )
            mv = work.tile([128, 2], f32, tag="mv", name="mv")
            nc.vector.bn_aggr(out=mv, in_=stats)
            rstd = work.tile([128, 1], f32, tag="rstd", name="rstd")
            nc.scalar.activation(
                out=rstd, in_=mv[:, 1:2], func=AF.Sqrt, bias=epst, scale=1.0
            )
            nc.vector.reciprocal(out=rstd, in_=rstd)
            return mv, rstd

        hT = [hera.tile([128, R], bf16, name=f"hT{ft}") for ft in range(FT)]
        h_res = [midp.tile([128, F], bf16, name=f"h{rb}") for rb in range(RB)]

        # ---------------- phase 1: adaLN-1 + attn-LN + transpose ----------------
        for rb in range(RB):
            x_rb = work.tile([128, F], f32, tag="x", name="x_rb")
            nc.scalar.dma_start(out=x_rb, in_=x_d[rb * 128 : (rb + 1) * 128, :])
            mv1, rstd1 = ln_stats(x_rb)
            nc.vector.scalar_tensor_tensor(
                out=x_rb,
                in0=x_rb,
                scalar=mv1[:, 0:1],
                in1=bc["Wa_bc"],
                op0=OP.subtract,
                op1=OP.mult,
            )
            nc.vector.scalar_tensor_tensor(
                out=h_res[rb],
                in0=x_rb,
                scalar=rstd1,
                in1=bc["Ba_bc"],
                op0=OP.mult,
                op1=OP.add,
            )
            mv2, rstd2 = ln_stats(h_res[rb])
            xn_bf = work.tile([128, F], bf16, tag="xn", name="xn_bf")
            nc.vector.tensor_scalar(
                out=xn_bf,
                in0=h_res[rb],
                scalar1=mv2[:, 0:1],
                scalar2=rstd2,
                op0=OP.subtract,
                op1=OP.mult,
            )
            for ft in range(FT):
                pt = ps_start.tile([128, 128], bf16, tag="tp", bufs=2, name="pt")
                nc.tensor.transpose(pt, xn_bf[:, ft * 128 : (ft + 1) * 128], ident)
                nc.scalar.activation(
                    out=hT[ft][:, rb * 128 : (rb + 1) * 128],
                    in_=pt,
                    func=AF.Identity,
                    bias=anbT[:, ft : ft + 1],
                    scale=anwT[:, ft : ft + 1],
                )

        # ---------------- phase 2: kv first (AllGather ASAP), then q ----------------
        pkv = ps_start.tile([128, 512], f32, tag="sp", bufs=3, name="pkv")
        for kt in range(FT):
            nc.tensor.matmul(
                pkv, wkv_sb[:, kt, :], hT[kt], start=(kt == 0), stop=(kt == FT - 1)
            )
        kvT_sb = work.tile([128, R], bf16, tag="kvT", bufs=1, name="kvT_sb")
        nc.scalar.activation(out=kvT_sb, in_=pkv, func=AF.Copy)

        kvT_bounce = dramp.tile([2 * D, R], bf16, name="kvT_bounce")
        kvT_all = dramp.tile([4 * 2 * D, R], bf16, name="kvT_all")
        nc.sync.dma_start(out=kvT_bounce, in_=kvT_sb)
        nc.gpsimd.collective_compute(
            "AllGather",
            OP.bypass,
            replica_groups=groups,
            ins=[kvT_bounce[:, :]],
            outs=[kvT_all[:, :]],
        )

        # q projection fills the AllGather wait
        qT = [attnp.tile([128, R], bf16, name=f"qT{mt}") for mt in range(MT)]
        for mt in range(MT):
            wqblk = wstr.tile([128, FT * 128], bf16, tag="wqb", bufs=3, name="wqblk")
            nc.scalar.dma_start(out=wqblk, in_=wq_d[mt])
            pq = ps_start.tile([128, 512], f32, tag="sp", bufs=3, name="pq")
            for kt in range(FT):
                nc.tensor.matmul(
                    pq,
                    wqblk[:, kt * 128 : (kt + 1) * 128],
                    hT[kt],
                    start=(kt == 0),
                    stop=(kt == FT - 1),
                )
            # fold the attention 1/sqrt(D)=0.125 scale into q
            nc.scalar.activation(out=qT[mt], in_=pq, func=AF.Copy, scale=0.125)

        # fold the out-proj bias into the residual while the AllGather is in
        # flight (x1 = px + (h_res + wo_b))
        for rb in range(RB):
            nc.vector.tensor_add(out=h_res[rb], in0=h_res[rb], in1=wob_bc)

        # preload exp's ACT table set during the AllGather window
        warm = work.tile([1, 1], f32, tag="warm", bufs=1, name="warm")
        nc.scalar.activation(out=warm, in_=epst[0:1, 0:1], func=AF.Exp)

        # prefetch wo (consumed in phase 5)
        wo2sb = [midp.tile([128, F], bf16, name=f"wo2_{hp}") for hp in range(MT)]
        for hp in range(MT):
            nc.gpsimd.dma_start(out=wo2sb[hp], in_=wo_d[hp * 128 : (hp + 1) * 128, :])

        # ---------------- phase 3: kT / v_ext assembly ----------------
        # kT_lo: k on partitions 0-63, zeros above  -> even heads (q on 0-63)
        # kT_hi: k on partitions 64-127, zeros below -> odd heads
        # Both give contraction-128 matmuls: no PE tile-mode switching.
        # v_ext dual layout [128, 192]:
        #   cols 0-63 = v dims, col 64 = ones, cols 65-127 = 0, cols 128-191 = v dims
        # even heads use lhsT [:,0:65]  -> po dims at partitions 0-63, den at 64
        # odd  heads use lhsT [:,64:192]-> po den at partition 0, dims at 64-127
        v_ext = [attnp.tile([128, 192], bf16, name=f"vext{kt}") for kt in range(KT)]
        for kt in range(KT):
            nc.vector.memset(v_ext[kt][:, 64:128], 0.0)
            nc.vector.memset(v_ext[kt][:, 64:65], 1.0)
        for r in range(4):
            vT_sb = work.tile([64, R], bf16, tag="vTs", bufs=1, name="vT_sb")
            nc.scalar.dma_start(
                out=vT_sb, in_=kvT_all[r * 128 + 64 : (r + 1) * 128, :]
            )
            for cc in range(4):
                kt = r * 4 + cc
                ptv = ps_start.tile([128, 128], bf16, tag="tp", bufs=2, name="ptv")
                nc.tensor.matmul(
                    ptv[:, 0:64],
                    vT_sb[:, cc * 128 : (cc + 1) * 128],
                    ident[0:64, 0:64],
                    is_transpose=True,
                )
                nc.vector.tensor_copy(out=v_ext[kt][:, 0:64], in_=ptv[:, 0:64])
                nc.vector.tensor_copy(out=v_ext[kt][:, 128:192], in_=ptv[:, 0:64])

        kT_lo = attnp.tile([128, T], bf16, name="kT_lo")
        kT_hi = attnp.tile([128, T], bf16, name="kT_hi")
        nc.vector.memset(kT_lo[64:128, :], 0.0)
        nc.vector.memset(kT_hi[0:64, :], 0.0)
        for r in range(4):
            nc.sync.dma_start(
                out=kT_lo[0:64, r * R : (r + 1) * R],
                in_=kvT_all[r * 128 : r * 128 + 64, :],
            )
            nc.gpsimd.dma_start(
                out=kT_hi[64:128, r * R : (r + 1) * R],
                in_=kvT_all[r * 128 : r * 128 + 64, :],
            )
        den16 = attnp.tile([16, R], f32, name="den16")
        rcpz = attnp.tile([128, R], bf16, name="rcpz")
        nc.vector.memset(rcpz, 0.0)

        cm_ps_start.__exit__(None, None, None)

        # attention-era PSUM: 3 x [128,1024] (6 banks) + 2 x po (2 banks)
        cm_ps_attn = tc.tile_pool(name="ps_attn", bufs=1, space="PSUM")
        ps_attn = cm_ps_attn.__enter__()

        # out2[hp2]: attention output stacked by head pair (head 2*hp2 on
        # partitions 0-63, head 2*hp2+1 on 64-127), raw (pre-normalization).
        out2 = [midp.tile([128, R], bf16, name=f"out2_{hp}") for hp in range(H // 2)]

        # ---------------- phase 4: attention ----------------
        for h in range(H):
            mt, even = h // 2, (h % 2) == 0
            kTs = kT_lo if even else kT_hi
            po = ps_attn.tile([128, 512], f32, tag="po", bufs=2, name="po")
            for c in range(KT // 2):
                psc = ps_attn.tile([128, 1024], f32, tag="ps2", bufs=3, name="psc")
                for half in range(2):
                    kt = 2 * c + half
                    nc.tensor.matmul(
                        psc[:, half * 512 : (half + 1) * 512],
                        kTs[:, kt * 128 : (kt + 1) * 128],
                        qT[mt],
                        start=True,
                        stop=True,
                    )
                pr = attnp.tile([128, 1024], bf16, tag="pr", bufs=4, name="pr")
                nc.scalar.activation(out=pr, in_=psc, func=AF.Exp)
                for half in range(2):
                    kt = 2 * c + half
                    lhs = v_ext[kt][:, 0:65] if even else v_ext[kt][:, 64:192]
                    outsl = po[0:65, :] if even else po[0:128, :]
                    nc.tensor.matmul(
                        outsl,
                        lhs,
                        pr[:, half * 512 : (half + 1) * 512],
                        start=(c == 0 and half == 0),
                        stop=(c == KT // 2 - 1 and half == 1),
                    )
            # stage the denominator row to SBUF (same-partition DVE copy),
            # then cross-partition SBUF->SBUF DMA into the den16 gather tile
            stg = work.tile([128, 512], f32, tag="dstg", name="dstg")
            if even:
                nc.vector.tensor_copy(out=stg[64:65, :], in_=po[64:65, :])
                nc.gpsimd.dma_start(out=den16[h : h + 1, :], in_=stg[64:65, :])
                nc.vector.tensor_copy(out=out2[mt][0:64, :], in_=po[0:64, :])
            else:
                nc.vector.tensor_copy(out=stg[0:1, :], in_=po[0:1, :])
                nc.gpsimd.dma_start(out=den16[h : h + 1, :], in_=stg[0:1, :])
                nc.vector.tensor_copy(out=out2[mt][64:128, :], in_=po[64:128, :])

        cm_ps_attn.__exit__(None, None, None)

        cm_ps_p5 = tc.tile_pool(name="ps_p5", bufs=1, space="PSUM")
        ps_p5 = cm_ps_p5.__enter__()

        # ---------------- phase 5: normalize + out proj + residual -> x1 ----------------
        with nc.allow_low_precision(reason="bf16 softmax denom broadcast"):
            nc.vector.reciprocal(out=rcpz[0:16, :], in_=den16)
        for hp2 in range(H // 2):
            bcrt = ps_p5.tile([128, 512], f32, tag="bcr", bufs=2, name="bcrt")
            nc.tensor.matmul(
                bcrt, sel2[:, hp2 * 128 : (hp2 + 1) * 128], rcpz, start=True, stop=True
            )
            nc.vector.tensor_mul(out=out2[hp2], in0=out2[hp2], in1=bcrt)

        x1 = [persist.tile([128, F], f32, name=f"x1_{rt}") for rt in range(RB)]
        for rt in range(RB):
            for fh in range(2):
                px = ps_p5.tile([128, 512], f32, tag="px", bufs=3, name="px")
                for hp2 in range(H // 2):
                    nc.tensor.matmul(
                        px,
                        out2[hp2][:, rt * 128 : (rt + 1) * 128],
                        wo2sb[hp2][:, fh * 512 : (fh + 1) * 512],
                        start=(hp2 == 0),
                        stop=(hp2 == H // 2 - 1),
                    )
                sl = slice(fh * 512, (fh + 1) * 512)
                nc.vector.tensor_add(out=x1[rt][:, sl], in0=px, in1=h_res[rt][:, sl])

        cm_ps_p5.__exit__(None, None, None)
        cm_mid.__exit__(None, None, None)
        cm_attn.__exit__(None, None, None)
        cm_hera.__exit__(None, None, None)

        cm_ps_p6 = tc.tile_pool(name="ps_p6", bufs=1, space="PSUM")
        ps_p6 = cm_ps_p6.__enter__()

        cm_mlps = tc.tile_pool(name="mlps", bufs=1)
        mlps = cm_mlps.__enter__()

        # ---------------- phase 6: adaLN-2 + transpose ----------------
        # LN core on DVE; Wf/Bf applied as per-partition scale/bias in the
        # PSUM->SBUF transpose copies. b2 is folded into x1 right after its
        # LN stats are consumed (y = mlp2 + (x1 + b2)).
        h2T = [mlps.tile([128, R], bf16, name=f"h2T{ft}") for ft in range(FT)]
        for rt in range(RB):
            mv3, rstd3 = ln_stats(x1[rt])
            xn_bf = work.tile([128, F], bf16, tag="xn", name="xn2_bf")
            nc.vector.tensor_scalar(
                out=xn_bf,
                in0=x1[rt],
                scalar1=mv3[:, 0:1],
                scalar2=rstd3,
                op0=OP.subtract,
                op1=OP.mult,
            )
            nc.vector.tensor_add(out=x1[rt], in0=x1[rt], in1=b2_bc)
            for ft in range(FT):
                pt = ps_p6.tile([128, 128], bf16, tag="tp2", bufs=2, name="pt2")
                nc.tensor.transpose(pt, xn_bf[:, ft * 128 : (ft + 1) * 128], ident)
                nc.scalar.activation(
                    out=h2T[ft][:, rt * 128 : (rt + 1) * 128],
                    in_=pt,
                    func=AF.Identity,
                    bias=BfC[:, ft : ft + 1],
                    scale=WfC[:, ft : ft + 1],
                )

        cm_ps_p6.__exit__(None, None, None)

        cm_ps_mlp = tc.tile_pool(name="ps_mlp", bufs=1, space="PSUM")
        ps_mlp = cm_ps_mlp.__enter__()

        # ---------------- phase 7: mlp1 + gelu ----------------
        g1T = [mlps.tile([128, R], bf16, name=f"g1T{mt}") for mt in range(MFT)]
        for mt in range(MFT):
            w1blk = wstr.tile([128, FT * 128], bf16, tag="w1b", bufs=3, name="w1blk")
            nc.gpsimd.dma_start(out=w1blk, in_=w1_d[mt])
            pg = ps_mlp.tile([128, 512], f32, tag="pg", bufs=3, name="pg")
            for kt in range(FT):
                nc.tensor.matmul(
                    pg,
                    w1blk[:, kt * 128 : (kt + 1) * 128],
                    h2T[kt],
                    start=(kt == 0),
                    stop=(kt == FT - 1),
                )
            nc.scalar.activation(
                out=g1T[mt],
                in_=pg,
                func=AF.Gelu,
                bias=b1_sb[:, mt : mt + 1],
                scale=1.0,
            )

        # ---------------- phase 8: mlp2 + residual -> y ----------------
        # mlp2's kt-th accumulation step only needs g1T[kt], so the fh=0
        # column sweep pipelines with mlp1 on the PE.
        for fh in range(2):
            pf = {}
            for rt in range(RB):
                pf[rt] = ps_mlp.tile(
                    [128, 512], f32, tag=f"pf{rt}", bufs=1, name=f"pf{rt}"
                )
            for kt in range(MFT):
                w2c = wstr.tile([128, 512], bf16, tag="w2c", bufs=6, name="w2c")
                eng = nc.gpsimd if kt % 2 == 0 else nc.sync
                eng.dma_start(
                    out=w2c, in_=w2_d[kt * 128 : (kt + 1) * 128, fh * 512 : (fh + 1) * 512]
                )
                for rt in range(RB):
                    nc.tensor.matmul(
                        pf[rt],
                        g1T[kt][:, rt * 128 : (rt + 1) * 128],
                        w2c,
                        start=(kt == 0),
                        stop=(kt == MFT - 1),
                    )
            for rt in range(RB):
                sl = slice(fh * 512, (fh + 1) * 512)
                yh = work.tile([128, 512], f32, tag="yh", bufs=2, name="yh")
                nc.vector.tensor_add(out=yh, in0=pf[rt], in1=x1[rt][:, sl])
                nc.sync.dma_start(out=y_d[rt * 128 : (rt + 1) * 128, sl], in_=yh)

        cm_ps_mlp.__exit__(None, None, None)
        cm_mlps.__exit__(None, None, None)

    nc.compile()
    return nc


def _prep_in_maps(inputs):
    f32 = np.float32
    wmod = np.concatenate(
        [inputs["amod_gw"], inputs["amod_bw"], inputs["fmod_gw"], inputs["fmod_bw"]],
        axis=1,
    ).astype(BF16)
    wq_t = np.ascontiguousarray(
        np.asarray(inputs["wq"]).astype(BF16).reshape(FT, 128, MT, 128)
        .transpose(2, 1, 0, 3).reshape(MT, 128, FT * 128)
    )
    w1_t = np.ascontiguousarray(
        np.asarray(inputs["w1"]).astype(BF16).reshape(FT, 128, MFT, 128)
        .transpose(2, 1, 0, 3).reshape(MFT, 128, FT * 128)
    )
    sel2 = np.zeros((128, H * 64), BF16)
    for hp2 in range(H // 2):
        sel2[2 * hp2, hp2 * 128 : hp2 * 128 + 64] = 1
        sel2[2 * hp2 + 1, hp2 * 128 + 64 : hp2 * 128 + 128] = 1
    # pack1: anwT | anbT | b1 columns
    pack1 = np.empty((128, 48), f32)
    pack1[:, 0:FT] = np.asarray(inputs["attn_nw"], f32).reshape(FT, 128).T
    pack1[:, FT : 2 * FT] = np.asarray(inputs["attn_nb"], f32).reshape(FT, 128).T
    pack1[:, 2 * FT :] = np.asarray(inputs["b1"], f32).reshape(MFT, 128).T
    # pack2: lnr chunks (amod_nw/nb, fmod_nw/nb) then modb chunks, [2,512] each
    pack2 = np.stack(
        [
            np.asarray(inputs[k], f32).reshape(2, 512)
            for k in (
                "amod_nw", "amod_nb", "fmod_nw", "fmod_nb",
                "amod_gb", "amod_bb", "fmod_gb", "fmod_bb",
            )
        ],
        axis=1,
    )  # [2, 8, 512]
    shared = dict(
        sel2=sel2,
        wmod=wmod,
        pack1=np.ascontiguousarray(pack1),
        pack2=np.ascontiguousarray(pack2),
        wq=wq_t,
        wkv=np.ascontiguousarray(
            np.asarray(inputs["wkv"]).astype(BF16).reshape(FT, 128, 2 * D)
            .transpose(1, 0, 2)
        ),
        wo=np.asarray(inputs["wo"]).astype(BF16),
        wo_bias=np.asarray(inputs["wo_b"]).astype(f32).reshape(1, F),
        w1=w1_t,
        w2=np.asarray(inputs["w2"]).astype(BF16),
        b2=np.asarray(inputs["b2"]).astype(f32).reshape(1, F),
    )
    x = np.asarray(inputs["x"]).astype(f32)
    cond = np.asarray(inputs["cond"]).astype(BF16)
    in_maps = []
    for c in range(NCORES):
        b, r0 = c // 4, (c % 4) * R
        m = dict(shared)
        m["x"] = np.ascontiguousarray(x[b, r0 : r0 + R, :])
        m["cond"] = np.ascontiguousarray(cond[b])
        in_maps.append(m)
    return in_maps


def _run(inputs, trace=False):
    from concourse.bass_utils import run_bass_kernel_spmd

    if "nc" not in _CACHE:
        _CACHE["nc"] = _build_nc()
    nc = _CACHE["nc"]
    in_maps = _prep_in_maps(inputs)
    res = run_bass_kernel_spmd(
        nc, in_maps, core_ids=list(range(NCORES)), trace=trace
    )
    y = np.empty((B, T, F), np.float32)
    for c in range(NCORES):
        b, r0 = c // 4, (c % 4) * R
        y[b, r0 : r0 + R, :] = res.results[c]["y"]
    return y, res


def kernel(**inputs) -> np.ndarray:
    y, _ = _run(inputs, trace=False)
    return y


if __name__ == "__main__":
    _build_nc()
    print("build OK")
